# revision 19
# baseline (speedup 1.0000x reference)
"""Cross-attention kernel for Trainium2, sharded over 8 NeuronCores.

Problem (hardcoded shapes): B=2, N=4096, M=1024, DIM=1024, H=16, D=64.
  q = rms_norm(x @ Wq.T + bq)        per-head, gamma gq, eps 1e-6
  k = rms_norm(ctx @ Wk.T + bk)      (Wk = first half of Wkv)
  v = ctx @ Wv.T + bv                (Wv = second half of Wkv)
  out = softmax(q k^T / sqrt(D) + mask_bias) @ v
  y = out @ Wo.T + bo
Sharding: 2 batches x 4 head-groups -> 8 cores.  Core c handles batch
c//4 and heads [4*(c%4), 4*(c%4)+4).  Host sums the 4 partial y's per
batch (row-sharded Wo) and adds bo.

Device-side design notes (v2):
 - Context compacted on host to valid tokens, padded to MC*128.
 - All activations/weights fp16 (fp32 PSUM).  exp(s) bounded by e^8.
 - Startup: small consts packed into 2 host tensors (2 DMA triggers
   instead of 7); x pre-tiled [P, NT, CC, 512] so each block slice is
   one contiguous descriptor set; x/wo triggers issued from the ACT
   queue (hwdge) in parallel with the sync queue to beat the ~635ns
   serial descriptor-gen cost per DMA trigger.
 - Main loop: per 512-query block, attention of block X-1 interleaves
   with Q-proj/rms of block X.  The reciprocal-broadcast (bcn) of
   block X-1 is issued mid block X (mi-pool psum, after scores mc2)
   so the PE in-order queue never waits on the DVE recip chain at a
   block boundary -- that stall was re-throttling HAM every block.
 - PV lags scores by 3 chunks so the acc-psum allocs of block X land
   after the normalize reads of block X-1 (no rotation stall).
 - Out-projection tail: 64 half-units [128,512] rotating through 8
   psum banks (2x big as halves + 3 acc + 1 mi), psum->sbuf copies
   alternating ACT/DVE, one merged [128,1024] DMA per 128-row tile
   (32 triggers instead of 64).
"""

import numpy as np

P = 128
B = 2
N = 4096
M = 1024
C = 1024  # DIM == COND_DIM
H = 16
D = 64
HC = 4  # heads per core
VD = HC * D  # 256 v/q/k dims per core
CC = C // P  # contraction chunks (8)
NT = N // 512  # query blocks of 512 (8)
QT = 2  # qdim tiles of 128 (VD / P)
EPS = 1e-6

_CACHE = {}


def _build(MC, dbg=False):
    """Build the kernel for MC context chunks of 128 (M_pad = 128*MC)."""
    key = ("nc", MC, dbg)
    if key in _CACHE:
        return _CACHE[key]

    import concourse.bass as bass  # noqa: F401
    import concourse.tile as tile
    from concourse import bacc, mybir

    f32 = mybir.dt.float32
    f16 = mybir.dt.float16
    AF = mybir.ActivationFunctionType
    MUL = mybir.AluOpType.mult
    MP = MC * P  # padded context length

    nc = bacc.Bacc("TRN2", target_bir_lowering=False, debug=False, num_devices=8)

    # All ACT functions used here (Exp, Ln, Copy, Identity) live in the
    # single table set "natural_log_exp_and_others".  The default set
    # assignment pass picks a different set per function and thrashes
    # ~20 ACT_TABLE_LOADs (~1.3us each); restrict the candidate list so
    # the fixpoint pass hoists ONE load to kernel entry.
    import types as _types
    import bass_rust as _bass_rust
    from concourse.hw_specs import get_activation_tables as _gat

    def _act_loads_single_set(self):
        has_act = any(
            isinstance(i, mybir.InstActivation)
            for b in self.main_func.blocks
            for i in b.instructions
        )
        if not has_act:
            return
        tables = list(_gat(self.m.arch).items())
        keep = "natural_log_exp_and_others"
        filtered = [(n, (set(fns) if n == keep else set())) for n, fns in tables]
        _bass_rust.insert_act_table_loads(self, filtered)

    nc.insert_act_table_loads = _types.MethodType(_act_loads_single_set, nc)

    # packed const widths, each slot 64-element aligned so matmul
    # stationary reads (FWL does 32-bit chunked fetches) stay aligned.
    W16_IND2 = 0
    W16_GQI = 64
    W16_GKI = W16_GQI + P
    W16_M16 = W16_GKI + P
    W16_IND34A = W16_M16 + 64
    W16_IND34B = W16_IND34A + 64
    W16_Z97 = W16_IND34B + 64
    W16_BVBM = W16_Z97 + 128
    W16 = W16_BVBM + MC * VD
    W32_BQ = 0
    W32_BK = 64
    W32_M32 = 128
    W32 = 192

    xt_d = nc.dram_tensor("xt", [P, NT, CC, 512], f16, kind="ExternalInput").ap()
    ctxt_d = nc.dram_tensor("ctxt", [P, CC, MP], f16, kind="ExternalInput").ap()
    wqt_d = nc.dram_tensor("wqt", [P, CC, VD], f16, kind="ExternalInput").ap()
    wkt_d = nc.dram_tensor("wkt", [P, CC, VD], f16, kind="ExternalInput").ap()
    wvt_d = nc.dram_tensor("wvt", [P, CC, VD], f16, kind="ExternalInput").ap()
    wot_d = nc.dram_tensor("wot", [P, QT, C], f16, kind="ExternalInput").ap()
    f16c_d = nc.dram_tensor("f16c", [P, W16], f16, kind="ExternalInput").ap()
    f32c_d = nc.dram_tensor("f32c", [P, W32], f32, kind="ExternalInput").ap()
    y_d = nc.dram_tensor("y", [N, C], f16, kind="ExternalOutput").ap()

    with tile.TileContext(nc) as tc:
        with (
            tc.tile_pool(name="consts", bufs=1) as consts,
            tc.tile_pool(name="xpool", bufs=1) as xpool,
            tc.tile_pool(name="kv", bufs=1) as kvp,
            tc.tile_pool(name="work", bufs=2) as work,
            tc.tile_pool(name="ptp", bufs=8) as ptp,
            tc.tile_pool(name="outp", bufs=1) as outp,
            # PSUM budget (8 banks): "big" = [128,1024] 2-bank tiles,
            # bufs=2 -> 4 banks (scores pairs, KV proj).  "acc" =
            # [128,512] 1-bank, bufs=3 -> 3 banks (pv01, pv23, den).
            # "mi" = [128,512] 1-bank, bufs=1 (Qproj / ss / qbc / bcn).
            tc.tile_pool(name="big", bufs=2, space="PSUM") as bigp,
            tc.tile_pool(name="acc", bufs=3, space="PSUM") as accp,
            tc.tile_pool(name="mi", bufs=1, space="PSUM") as mip,
        ):
            # ---- input DMA: batched triggers, split across the two
            # hwdge queues (sync + ACT) so descriptor-gen parallelizes.
            f32c_sb = consts.tile([P, W32], f32)
            nc.sync.dma_start(f32c_sb[:], f32c_d[:])
            f16c_sb = consts.tile([P, W16], f16)
            nc.sync.dma_start(f16c_sb[:], f16c_d[:])
            ctx_sb = xpool.tile([P, CC, MP], f16)
            h1 = CC // 2
            nc.sync.dma_start(ctx_sb[:, 0:h1, :], ctxt_d[:, 0:h1, :])
            nc.sync.dma_start(ctx_sb[:, h1:CC, :], ctxt_d[:, h1:CC, :])
            wk_sb = consts.tile([P, CC, VD], f16)
            nc.sync.dma_start(wk_sb[:], wkt_d[:])
            wv_sb = consts.tile([P, CC, VD], f16)
            nc.sync.dma_start(wv_sb[:], wvt_d[:])
            wq_sb = consts.tile([P, CC, VD], f16)
            nc.sync.dma_start(wq_sb[:], wqt_d[:])
            # x blocks + wo follow on the same queue (consumption order;
            # a second parallel queue would steal HBM bandwidth from the
            # critical ctx/wk path)
            xt_sb = xpool.tile([P, NT, CC, 512], f16)
            for nt_ in range(NT):
                nc.sync.dma_start(xt_sb[:, nt_, :, :], xt_d[:, nt_, :, :])
            wo_sb = consts.tile([P, QT, C], f16)
            nc.sync.dma_start(wo_sb[:], wot_d[:])

            # views into the const packs
            ind2_sb = f16c_sb[:, W16_IND2 : W16_IND2 + 2]
            gqi_sb = f16c_sb[:, W16_GQI : W16_GQI + P]
            gki_sb = f16c_sb[:, W16_GKI : W16_GKI + P]
            m16_sb = f16c_sb[:, W16_M16 : W16_M16 + MC]
            bq_sb = f32c_sb[:, W32_BQ : W32_BQ + QT]
            bk_sb = f32c_sb[:, W32_BK : W32_BK + QT]
            m32_sb = f32c_sb[:, W32_M32 : W32_M32 + MC]

            def bvbm_sl(mc):
                return f16c_sb[:, W16_BVBM + mc * VD : W16_BVBM + (mc + 1) * VD]

            # 34-col ss stationaries: S0 puts tile-0 sums at rows {0,1},
            # S1 puts tile-1 sums at rows {32,33}; all other rows get an
            # explicit 0 write (accumulated) so the [0:34] Ln read below
            # never touches stale psum rows.
            ind34a_sb = f16c_sb[:, W16_IND34A : W16_IND34A + 34]
            ind34b_sb = f16c_sb[:, W16_IND34B : W16_IND34B + 34]
            z97_sb = f16c_sb[:, W16_Z97 : W16_Z97 + 97]

            eps_sb = consts.tile([P, 1], f32)
            nc.vector.memset(eps_sb[:], EPS)
            ones64_sb = consts.tile([P, 64], f16)
            nc.vector.memset(ones64_sb[:], 1.0)

            # ================= KV phase =================
            # K projection: out [kdim, m] (2 tiles of 128 kdims)
            ktn = [kvp.tile([P, MP], f16, name=f"ktn{t}") for t in range(QT)]
            kraw = [kvp.tile([P, MP], f16, name=f"kraw{t}") for t in range(QT)]
            for t in range(QT):
                ps_k = bigp.tile([P, 1024], f32, tag="big")
                for cc in range(CC):
                    for ms in range(0, MP, 512):
                        me = min(ms + 512, MP)
                        nc.tensor.matmul(
                            ps_k[:, ms:me],
                            wk_sb[:, cc, t * P : (t + 1) * P],
                            ctx_sb[:, cc, ms:me],
                            start=(cc == 0),
                            stop=(cc == CC - 1),
                        )
                nc.vector.tensor_scalar_add(
                    kraw[t][:], ps_k[:, :MP], bk_sb[:, t : t + 1]
                )
                sq = work.tile([P, MP], f16, tag="ksq", name="ksq")
                nc.vector.tensor_mul(sq[:], kraw[t][:], kraw[t][:])
                rsl = slice(32 * t, 32 * t + 2)
                ps_ss = bigp.tile([P, 1024], f32, tag="big", name=f"kss{t}")
                for ms in range(0, MP, 512):
                    me = min(ms + 512, MP)
                    nc.tensor.matmul(
                        ps_ss[rsl, ms:me],
                        ind2_sb[:],
                        sq[:, ms:me],
                        start=True,
                        stop=True,
                    )
                # rsqrt(mean_sq + eps) = Exp(-0.5 * Ln(ss/D + eps)); Ln and
                # Exp share one ACT table set so no table switches ever.
                srt = work.tile([34, MP], f32, tag="ksrt", name="ksrt", bufs=1)
                nc.scalar.activation(
                    srt[rsl, :], ps_ss[rsl, :MP], AF.Ln, scale=1.0 / D,
                    bias=eps_sb[rsl, :],
                )
                rstd16 = work.tile([34, MP], f16, tag="krstd16", name="krstd16")
                nc.scalar.activation(rstd16[rsl, :], srt[rsl, :], AF.Exp, scale=-0.5)
                ps_bc = bigp.tile([P, 1024], f32, tag="big", name=f"kbc{t}")
                for ms in range(0, MP, 512):
                    me = min(ms + 512, MP)
                    nc.tensor.matmul(
                        ps_bc[:, ms:me],
                        gki_sb[rsl, :],
                        rstd16[rsl, ms:me],
                        start=True,
                        stop=True,
                    )
                nc.vector.tensor_mul(ktn[t][:], kraw[t][:], ps_bc[:, :MP])

            # V projection directly in [m, vdim] layout + bias + mask
            vt = []
            for mc in range(MC):
                pool = mip if mc % 2 == 0 else accp
                ps_v = pool.tile(
                    [P, 512], f32, tag=("mi" if mc % 2 == 0 else "acc"),
                    name=f"v{mc}",
                )
                for cc in range(CC):
                    nc.tensor.matmul(
                        ps_v[:, 0:VD],
                        ctx_sb[:, cc, mc * P : (mc + 1) * P],
                        wv_sb[:, cc, :],
                        start=(cc == 0),
                        stop=(cc == CC - 1),
                    )
                vtile = kvp.tile([P, VD], f16, name=f"vt{mc}")
                # v = vproj * maskcol + (bv * maskcol)
                nc.vector.scalar_tensor_tensor(
                    out=vtile[:],
                    in0=ps_v[:, 0:VD],
                    scalar=m32_sb[:, mc : mc + 1],
                    in1=bvbm_sl(mc),
                    op0=MUL,
                    op1=mybir.AluOpType.add,
                )
                vt.append(vtile)

            # ================= main pipelined loop =================
            # Per iteration `step`:
            #   - attention of block ant = step-1 (scores/exp/pv/den)
            #   - Q-proj + rms + qtn finalize of block `step`
            #   - normalize (bcn broadcast + outtn stt) of block step-2,
            #     issued after scores(mc2) so the PE queue never waits
            #     on the DVE recip chain at a block boundary.
            outtn = [
                outp.tile([P, N], f16, name=f"outtn{t}") for t in range(QT)
            ]

            # slot schedule within an iteration (keys: mc slot index)
            QW_SLOT = {1: [0, 1], 2: [2, 3], 3: [4]}
            PV_SLOT = {3: [0, 1], 4: [2], 5: [3, 4]}
            BCN_SLOT = 2
            QBC_SLOT = 4
            if MC < 5:
                # degenerate masks: fall back to simple spread
                QW_SLOT = {i: [i] for i in range(min(MC, 5))}
                for i in range(min(MC, 5), 5):
                    QW_SLOT.setdefault(MC, []).append(i)
                PV_SLOT = {MC: list(range(MC))}
                BCN_SLOT = min(2, MC)
                QBC_SLOT = min(4, MC)

            qstate = [None]   # (raw16, sq16, qtn) of block `step`
            yh_state = [0, None]  # [next half index, current y_sb tile]

            def y_half_unit(on_act=False):
                """One out-proj half [128,512] through the mi chain."""
                h = yh_state[0]
                yh_state[0] = h + 1
                tcn, half = divmod(h, 2)
                tsl = slice(tcn * P, (tcn + 1) * P)
                ysl = slice(half * 512, (half + 1) * 512)
                ps_y = mip.tile([P, 512], f32, tag="mi", name="ps_y")
                for t in range(QT):
                    nc.tensor.matmul(
                        ps_y[:],
                        outtn[t][:, tsl],
                        wo_sb[:, t, ysl],
                        start=(t == 0),
                        stop=(t == QT - 1),
                    )
                if half == 0:
                    yh_state[1] = work.tile(
                        [P, 1024], f16, tag="ysb", name="ysb", bufs=4
                    )
                y_sb = yh_state[1]
                if on_act:
                    nc.scalar.activation(y_sb[:, ysl], ps_y, AF.Copy)
                else:
                    nc.vector.tensor_copy(y_sb[:, ysl], ps_y)
                if half == 1:
                    nc.sync.dma_start(y_d[tsl, :], y_sb[:])

            r16_state = [None]
            aqtn_state = [None]  # qtn tiles of the block being attended
            # pending normalize: (pr-> ps_pv tiles, ps_den, rd16, ansl)
            norm_state = [None]

            def qwork_slice(step, raw16, sq16, i):
                """Issue the i-th slice of block `step`'s Q-proj/rms."""
                nsl_t = step  # xt_sb block index
                if i in (0, 1, 2, 3):
                    t, piece = divmod(i, 2)
                    if piece == 0:
                        qps[0] = mip.tile([P, 512], f32, tag="mi", name=f"q{t}")
                    ps_q = qps[0]
                    for cc in range(4 * piece, 4 * piece + 4):
                        nc.tensor.matmul(
                            ps_q[:],
                            wq_sb[:, cc, t * P : (t + 1) * P],
                            xt_sb[:, nsl_t, cc, :],
                            start=(cc == 0),
                            stop=(cc == CC - 1),
                        )
                    if piece == 1:
                        nc.vector.tensor_scalar_add(
                            raw16[t][:], ps_q[:], bq_sb[:, t : t + 1]
                        )
                        nc.vector.tensor_mul(
                            sq16[t][:], raw16[t][:], raw16[t][:]
                        )
                elif i == 4:
                    # ss pair: rows 0:2 (tile0) and 32:34 (tile1); middle
                    # rows written 0 so the Ln read of [0:34] is race-free.
                    ps_ss = mip.tile([P, 512], f32, tag="mi", name="qss")
                    nc.tensor.matmul(
                        ps_ss[0:34, :], ind34a_sb, sq16[0][:],
                        start=True, stop=False,
                    )
                    nc.tensor.matmul(
                        ps_ss[0:34, :], ind34b_sb, sq16[1][:],
                        start=False, stop=True,
                    )
                    srt = work.tile([34, 512], f32, tag="qsrt", name="qsrt")
                    nc.scalar.activation(
                        srt[:], ps_ss[0:34, :], AF.Ln, scale=1.0 / D,
                        bias=eps_sb[0:34, :],
                    )
                    r16 = work.tile([34, 512], f16, tag="qr16", name="qr16")
                    nc.scalar.activation(r16[:], srt[:], AF.Exp, scale=-0.5)
                    r16_state[0] = r16

            def issue_norm():
                """bcn broadcast + outtn normalize of the pending block.

                The per-(head, query) reciprocal rows {0,32,64,96} of rd16
                are broadcast across each head's 64 v-dim partitions on the
                (otherwise idle) GPSIMD engine -- no PE or PSUM involved,
                so the PE queue never waits on this chain.
                """
                if norm_state[0] is None:
                    return
                ps_pv_p, ps_den_p, rd16_p, ansl_p = norm_state[0]
                norm_state[0] = None
                for pr in range(2):
                    ps_bcn = mip.tile([P, 512], f32, tag="mi", name=f"bcn{pr}")
                    for hh in range(2):
                        h = 2 * pr + hh
                        nc.tensor.matmul(
                            ps_bcn[64 * hh : 64 * hh + 64, :],
                            ones64_sb[32 * h : 32 * h + 1, :],
                            rd16_p[32 * h : 32 * h + 1, :],
                            start=True,
                            stop=True,
                            tile_position=(32 * h, 64 * hh),
                            skip_group_check=True,
                        )
                    bcn_sb = work.tile(
                        [P, 512], f16, tag=f"bcn{pr}", name=f"bcn{pr}"
                    )
                    nc.vector.tensor_copy(bcn_sb[:], ps_bcn[:])
                    # ps_den holds den/256 (mask stationary is 1/256); the
                    # stt scalar 1/256 compensates exactly.
                    nc.vector.scalar_tensor_tensor(
                        out=outtn[pr][:, ansl_p],
                        in0=ps_pv_p[pr][:],
                        scalar=1.0 / 256.0,
                        in1=bcn_sb[:],
                        op0=MUL,
                        op1=MUL,
                    )

            for step in range(NT + 1):
                do_q = step < NT
                do_attn = step > 0
                ant = step - 1  # attention block index

                if do_q:
                    raw16 = [
                        work.tile([P, 512], f16, tag=f"qraw{t}", name=f"qraw{t}")
                        for t in range(QT)
                    ]
                    sq16 = [
                        work.tile([P, 512], f16, tag=f"qsq{t}", name=f"qsq{t}")
                        for t in range(QT)
                    ]
                    qtn_tiles = [
                        work.tile([P, 512], f16, tag=f"qtn{t}", name=f"qtn{t}")
                        for t in range(QT)
                    ]
                    qstate[0] = (raw16, sq16, qtn_tiles)
                else:
                    raw16 = sq16 = qtn_tiles = None

                qps = [None]
                aqtn = aqtn_state[0]

                if do_attn:
                    ansl = slice(ant * 512, (ant + 1) * 512)
                    pt_tiles = {}
                    ps_pv = None
                    ps_den = None

                n_slots = (MC + 1) if do_attn else (MC + 1)
                qi_sched = QW_SLOT if do_q else {}

                for mc in range(n_slots):
                    # scores for both head pairs, row-tiled (K=64)
                    if do_attn and mc < MC:
                        pt_pair = []
                        for pr in range(2):
                            ps_s = bigp.tile(
                                [P, 1024], f32, tag="big", name=f"s{mc}_{pr}"
                            )
                            kt = ktn[pr]
                            qt = aqtn[pr]
                            msl = slice(mc * P, (mc + 1) * P)
                            nc.tensor.matmul(
                                ps_s[:, 0:512], kt[0:64, msl], qt[0:64, :],
                                start=True, stop=True,
                            )
                            nc.tensor.matmul(
                                ps_s[:, 512:1024], kt[64:128, msl], qt[64:128, :],
                                start=True, stop=True,
                            )
                            pt = ptp.tile([P, 1024], f16, tag="pt")
                            nc.scalar.activation(pt[:], ps_s[:], AF.Exp)
                            pt_pair.append(pt)
                        pt_tiles[mc] = pt_pair

                    # deferred normalize of block ant-1 (or step-2)
                    if mc == BCN_SLOT:
                        issue_norm()

                    # Q-proj slices of block `step`
                    if do_q:
                        for i in qi_sched.get(mc, []):
                            qwork_slice(step, raw16, sq16, i)

                    # qtn finalize of block `step` (uses this step's r16).
                    # t=0 before the pv group, t=1 after -- the pv matmuls
                    # between them cover the DVE read of the shared mi bank
                    # so the PE queue doesn't micro-stall.
                    def qbc(t):
                        r16_cur = r16_state[0]
                        ps_bc = mip.tile([P, 512], f32, tag="mi", name=f"qbc{t}")
                        nc.tensor.matmul(
                            ps_bc[:],
                            gqi_sb[32 * t : 32 * t + 2, :],
                            r16_cur[32 * t : 32 * t + 2, :],
                            start=True,
                            stop=True,
                        )
                        nc.vector.tensor_mul(
                            qtn_tiles[t][:], raw16[t][:], ps_bc[:]
                        )

                    if do_q and mc == QBC_SLOT:
                        qbc(0)

                    # pv/den groups (lag-3 behind scores)
                    if do_attn:
                        for pmc in PV_SLOT.get(mc, []):
                            if pmc >= MC:
                                continue
                            if ps_pv is None:
                                ps_pv = [
                                    accp.tile(
                                        [P, 512], f32, tag="acc", name=f"pv{pr}"
                                    )
                                    for pr in range(2)
                                ]
                                ps_den = accp.tile(
                                    [P, 512], f32, tag="acc", name="den"
                                )
                                # zero-fill rows 0:97 so the [0:97] recip
                                # read below never touches stale psum.
                                nc.tensor.matmul(
                                    ps_den[0:97, :],
                                    z97_sb,
                                    ktn[0][0:128, 0:512],
                                    start=True,
                                    stop=False,
                                    skip_group_check=True,
                                )
                            pt_pair = pt_tiles[pmc]
                            for pr in range(2):
                                pt = pt_pair[pr]
                                for hh in range(2):
                                    h = 2 * pr + hh
                                    nc.tensor.matmul(
                                        ps_pv[pr][64 * hh : 64 * hh + 64, :],
                                        vt[pmc][:, 64 * h : 64 * h + 64],
                                        pt[:, 512 * hh : 512 * hh + 512],
                                        start=(pmc == 0),
                                        stop=(pmc == MC - 1),
                                        tile_position=(0, 64 * hh),
                                        skip_group_check=True,
                                    )
                            for pr in range(2):
                                pt = pt_pair[pr]
                                for hh in range(2):
                                    h = 2 * pr + hh
                                    nc.tensor.matmul(
                                        ps_den[32 * h : 32 * h + 1, :],
                                        m16_sb[:, pmc : pmc + 1],
                                        pt[:, 512 * hh : 512 * hh + 512],
                                        start=False,
                                        stop=(pmc == MC - 1),
                                        tile_position=(0, 32 * h),
                                        skip_group_check=True,
                                    )

                    if do_q and mc == QBC_SLOT:
                        qbc(1)

                    # out-proj half-units for completed query blocks ride the
                    # loop's engine slack (PE +0.9us/blk, DVE copies, sync DMA).
                    # Only blocks whose normalize finished LAST iter (< ant-1)
                    # are eligible, so the PE queue never waits on an stt.
                    if (
                        do_attn
                        and mc in (1, MC)
                        and yh_state[0] < 8 * (ant - 1)
                        and yh_state[0] < 2 * (ant - 1)
                    ):
                        y_half_unit()

                # end of iteration: recip of this block's den; normalize
                # itself is deferred into the next iteration (BCN_SLOT).
                if do_attn:
                    rd32 = work.tile([P, 512], f32, tag="rd32", name="rd32")
                    nc.vector.reciprocal_approx_fast(
                        out=rd32[0:97, :], in_=ps_den[0:97, :]
                    )
                    rd16 = work.tile([P, 512], f16, tag="rd16", name="rd16")
                    nc.vector.tensor_copy(rd16[0:97, :], rd32[0:97, :])
                    norm_state[0] = (ps_pv, ps_den, rd16, ansl)

                if do_q:
                    aqtn_state[0] = qtn_tiles

            # normalize of the final block
            issue_norm()

            # ================= output projection =================
            # 64 half-units [128,512] rotating through 8 psum banks:
            # groups of 8 halves = [bigA.lo, bigA.hi, acc0, acc1, acc2,
            # mi, bigB.lo, bigB.hi].  Copies alternate ACT/DVE; one
            # merged [128,1024] DMA per 128-row tile (32 triggers).
            n_tiles = N // P  # 32

            def slot_cycle():
                """Infinite cycle of tail psum slots: ('big', tile-or-None)
                pairs where big tiles carry 2 halves with one wide copy."""
                while True:
                    bigA = bigp.tile([P, 1024], f32, tag="big", name="ytA")
                    yield ("bigpair", bigA)
                    for a in range(3):
                        acc_t = accp.tile(
                            [P, 512], f32, tag="acc", name=f"yacc{a}"
                        )
                        yield ("single", acc_t)
                    mi_t = mip.tile([P, 512], f32, tag="mi", name="ymi")
                    yield ("single", mi_t)
                    bigB = bigp.tile([P, 1024], f32, tag="big", name="ytB")
                    yield ("bigpair", bigB)

            slots = slot_cycle()
            h = yh_state[0]
            assert h % 2 == 0
            y_sb_cur = [None]
            ns = 0  # single-slot counter for copy-engine balance
            while h < 2 * n_tiles:
                kind, tile_ = next(slots)
                if kind == "bigpair" and h + 1 < 2 * n_tiles and h % 2 == 0:
                    tcn = h // 2
                    tsl = slice(tcn * P, (tcn + 1) * P)
                    for half in range(2):
                        ysl = slice(half * 512, (half + 1) * 512)
                        for t in range(QT):
                            nc.tensor.matmul(
                                tile_[:, ysl],
                                outtn[t][:, tsl],
                                wo_sb[:, t, ysl],
                                start=(t == 0),
                                stop=(t == QT - 1),
                            )
                    y_sb = work.tile(
                        [P, 1024], f16, tag="ysb", name="ysb", bufs=4
                    )
                    # one wide ACT copy for both halves
                    nc.scalar.activation(y_sb[:], tile_[:], AF.Copy)
                    eng = nc.sync if (tcn % 2 == 0) else nc.scalar
                    eng.dma_start(y_d[tsl, :], y_sb[:])
                    h += 2
                else:
                    if kind == "bigpair":
                        tile_ = tile_[:, 0:512]
                    else:
                        tile_ = tile_[:]
                    tcn, half = divmod(h, 2)
                    tsl = slice(tcn * P, (tcn + 1) * P)
                    ysl = slice(half * 512, (half + 1) * 512)
                    for t in range(QT):
                        nc.tensor.matmul(
                            tile_,
                            outtn[t][:, tsl],
                            wo_sb[:, t, ysl],
                            start=(t == 0),
                            stop=(t == QT - 1),
                        )
                    if half == 0:
                        y_sb_cur[0] = work.tile(
                            [P, 1024], f16, tag="ysb", name="ysb", bufs=4
                        )
                    y_sb = y_sb_cur[0]
                    nc.vector.tensor_copy(y_sb[:, ysl], tile_)
                    ns += 1
                    if half == 1:
                        eng = nc.sync if (tcn % 2 == 0) else nc.scalar
                        eng.dma_start(y_d[tsl, :], y_sb[:])
                    h += 1

    nc.compile()
    _CACHE[key] = nc
    return nc


def _prep(x, context, context_mask, Wq, bq, Wkv, bkv, gq, gk, Wo, bo):
    """Host-side: compaction, transposes, per-core weight slices."""
    f16 = np.float16
    f32 = np.float32
    mask = np.asarray(context_mask)
    idxs = [np.nonzero(mask[b])[0] for b in range(B)]
    mv = [len(ix) for ix in idxs]
    MC = max(1, (max(mv) + P - 1) // P)
    MP = MC * P

    # compacted, padded, transposed context per batch (fp16)
    ctxt = []
    for b in range(B):
        cc = np.zeros((MP, C), dtype=f32)
        cc[: mv[b]] = np.asarray(context[b], dtype=f32)[idxs[b]]
        ctxt.append(np.ascontiguousarray(cc.T, dtype=f16))

    # mask columns [128, MC] per batch
    m32 = []
    for b in range(B):
        m = np.zeros((MP,), dtype=f32)
        m[: mv[b]] = 1.0
        m32.append(np.ascontiguousarray(m.reshape(MC, P).T))

    def perm(a, p):
        """[(o p), m...] -> [p, o, m...] contiguous, fp16."""
        a = np.asarray(a, dtype=f32)
        o = a.shape[0] // p
        return np.ascontiguousarray(
            a.reshape(o, p, *a.shape[1:]).swapaxes(0, 1), dtype=f16
        )

    xt = []
    for b in range(B):
        xb = perm(np.asarray(x[b], dtype=f32).T, P)  # [P, CC, N]
        xt.append(
            np.ascontiguousarray(
                xb.reshape(P, CC, NT, 512).transpose(0, 2, 1, 3)
            )
        )  # [P, NT, CC, 512]
    ctxt = [perm(cc, P) for cc in ctxt]

    ind2 = np.zeros((P, 2), dtype=f16)
    ind2[0:64, 0] = 1.0
    ind2[64:128, 1] = 1.0
    ind34 = np.zeros((P, 68), dtype=f16)
    ind34[0:64, 0] = 1.0
    ind34[64:128, 1] = 1.0
    ind34[0:64, 34 + 32] = 1.0
    ind34[64:128, 34 + 33] = 1.0

    Wq = np.asarray(Wq, dtype=f32)
    Wkv = np.asarray(Wkv, dtype=f32)
    Wo = np.asarray(Wo, dtype=f32)
    bq = np.asarray(bq, dtype=f32)
    bkv = np.asarray(bkv, dtype=f32)
    gq = np.asarray(gq, dtype=f32)
    gk = np.asarray(gk, dtype=f32)

    in_maps = []
    for c in range(8):
        bi, hg = c // 4, c % 4
        hs = slice(VD * hg, VD * (hg + 1))  # 256 dims for 4 heads
        heads = [hg * HC + j for j in range(HC)]

        gqi = np.zeros((P, P), dtype=f16)
        gki = np.zeros((P, P), dtype=f16)
        for t in range(QT):
            for j in range(2):
                h = heads[2 * t + j]
                gqi[32 * t + j, 64 * j : 64 * j + 64] = (
                    gq[h] * (1.0 / np.sqrt(D))
                ).astype(f16)
                gki[32 * t + j, 64 * j : 64 * j + 64] = gk[h].astype(f16)

        bv = bkv[C + VD * hg : C + VD * (hg + 1)]
        bvbm = np.zeros((P, MC, VD), dtype=f16)
        for mc in range(MC):
            bvbm[:, mc, :] = (
                m32[bi][:, mc : mc + 1] * bv[None, :]
            ).astype(f16)

        # packed const tensors (offsets must match device-side slots)
        W16_BVBM = 64 + 128 + 128 + 64 + 64 + 64 + 128
        f16c = np.zeros((P, W16_BVBM + MC * VD), dtype=f16)
        f16c[:, 0:2] = ind2
        f16c[:, 64 : 64 + 128] = gqi
        f16c[:, 192 : 192 + 128] = gki
        f16c[:, 320 : 320 + MC] = (m32[bi] / 256.0).astype(f16)
        f16c[:, 384 : 384 + 34] = ind34[:, 0:34]
        f16c[:, 448 : 448 + 34] = ind34[:, 34:68]
        # z97 slot at 512 stays zero
        f16c[:, W16_BVBM :] = bvbm.reshape(P, MC * VD)
        f32c = np.zeros((P, 192), dtype=f32)
        f32c[:, 0:QT] = np.ascontiguousarray(bq[hs].reshape(QT, P).T)
        f32c[:, 64 : 64 + QT] = np.ascontiguousarray(bkv[hs].reshape(QT, P).T)
        f32c[:, 128 : 128 + MC] = m32[bi]

        in_maps.append(
            {
                "xt": xt[bi],
                "ctxt": ctxt[bi],
                "wqt": perm(Wq[hs].T, P),
                "wkt": perm(Wkv[hs].T, P),
                "wvt": perm(Wkv[C + VD * hg : C + VD * (hg + 1)].T, P),
                "wot": perm(Wo[:, hs].T, P),
                "f16c": np.ascontiguousarray(f16c),
                "f32c": np.ascontiguousarray(f32c),
            }
        )
    return in_maps, MC


def _run(in_maps, MC, **spmd_kwargs):
    from concourse import bass_utils

    nc = _build(MC)
    return bass_utils.run_bass_kernel_spmd(
        nc, in_maps, core_ids=list(range(8)), **spmd_kwargs
    )


def kernel(x, context, context_mask, Wq, bq, Wkv, bkv, gq, gk, Wo, bo):
    in_maps, MC = _prep(
        x, context, context_mask, Wq, bq, Wkv, bkv, gq, gk, Wo, bo
    )
    res = _run(in_maps, MC)
    y = np.zeros((B, N, C), dtype=np.float64)
    for c in range(8):
        y[c // 4] += res.results[c]["y"].astype(np.float64)
    y += np.asarray(bo, dtype=np.float64)[None, None, :]
    return y.astype(np.float32)


# revision 20
# speedup vs baseline: 1.0346x; 1.0346x over previous
"""Cross-attention kernel for Trainium2, sharded over 8 NeuronCores.

Problem (hardcoded shapes): B=2, N=4096, M=1024, DIM=1024, H=16, D=64.
  q = rms_norm(x @ Wq.T + bq)        per-head, gamma gq, eps 1e-6
  k = rms_norm(ctx @ Wk.T + bk)      (Wk = first half of Wkv)
  v = ctx @ Wv.T + bv                (Wv = second half of Wkv)
  out = softmax(q k^T / sqrt(D) + mask_bias) @ v
  y = out @ Wo.T + bo
Sharding: 2 batches x 4 head-groups -> 8 cores.  Core c handles batch
c//4 and heads [4*(c%4), 4*(c%4)+4).  Host sums the 4 partial y's per
batch (row-sharded Wo) and adds bo.

Device-side design notes (v2):
 - Context compacted on host to valid tokens, padded to MC*128.
 - All activations/weights fp16 (fp32 PSUM).  exp(s) bounded by e^8.
 - Startup: small consts packed into 2 host tensors (2 DMA triggers
   instead of 7); x pre-tiled [P, NT, CC, 512] so each block slice is
   one contiguous descriptor set; x/wo triggers issued from the ACT
   queue (hwdge) in parallel with the sync queue to beat the ~635ns
   serial descriptor-gen cost per DMA trigger.
 - Main loop: per 512-query block, attention of block X-1 interleaves
   with Q-proj/rms of block X.  The reciprocal-broadcast (bcn) of
   block X-1 is issued mid block X (mi-pool psum, after scores mc2)
   so the PE in-order queue never waits on the DVE recip chain at a
   block boundary -- that stall was re-throttling HAM every block.
 - PV lags scores by 3 chunks so the acc-psum allocs of block X land
   after the normalize reads of block X-1 (no rotation stall).
 - Out-projection tail: 64 half-units [128,512] rotating through 8
   psum banks (2x big as halves + 3 acc + 1 mi), psum->sbuf copies
   alternating ACT/DVE, one merged [128,1024] DMA per 128-row tile
   (32 triggers instead of 64).
"""

import numpy as np

P = 128
B = 2
N = 4096
M = 1024
C = 1024  # DIM == COND_DIM
H = 16
D = 64
HC = 4  # heads per core
VD = HC * D  # 256 v/q/k dims per core
CC = C // P  # contraction chunks (8)
NT = N // 512  # query blocks of 512 (8)
QT = 2  # qdim tiles of 128 (VD / P)
EPS = 1e-6

_CACHE = {}


def _build(MC, dbg=False):
    """Build the kernel for MC context chunks of 128 (M_pad = 128*MC)."""
    key = ("nc", MC, dbg)
    if key in _CACHE:
        return _CACHE[key]

    import concourse.bass as bass  # noqa: F401
    import concourse.tile as tile
    from concourse import bacc, mybir

    f32 = mybir.dt.float32
    f16 = mybir.dt.float16
    AF = mybir.ActivationFunctionType
    MUL = mybir.AluOpType.mult
    MP = MC * P  # padded context length

    nc = bacc.Bacc("TRN2", target_bir_lowering=False, debug=False, num_devices=8)

    # All ACT functions used here (Exp, Ln, Copy, Identity) live in the
    # single table set "natural_log_exp_and_others".  The default set
    # assignment pass picks a different set per function and thrashes
    # ~20 ACT_TABLE_LOADs (~1.3us each); restrict the candidate list so
    # the fixpoint pass hoists ONE load to kernel entry.
    import types as _types
    import bass_rust as _bass_rust
    from concourse.hw_specs import get_activation_tables as _gat

    def _act_loads_single_set(self):
        has_act = any(
            isinstance(i, mybir.InstActivation)
            for b in self.main_func.blocks
            for i in b.instructions
        )
        if not has_act:
            return
        tables = list(_gat(self.m.arch).items())
        keep = "natural_log_exp_and_others"
        filtered = [(n, (set(fns) if n == keep else set())) for n, fns in tables]
        _bass_rust.insert_act_table_loads(self, filtered)

    nc.insert_act_table_loads = _types.MethodType(_act_loads_single_set, nc)

    # packed const widths, each slot 64-element aligned so matmul
    # stationary reads (FWL does 32-bit chunked fetches) stay aligned.
    W16_IND2 = 0
    W16_GQI = 64
    W16_GKI = W16_GQI + P
    W16_M16 = W16_GKI + P
    W16_IND34A = W16_M16 + 64
    W16_IND34B = W16_IND34A + 64
    W16_Z97 = W16_IND34B + 64
    W16_BVBM = W16_Z97 + 128
    W16 = W16_BVBM + MC * VD
    W32_BQ = 0
    W32_BK = 64
    W32_M32 = 128
    W32 = 192

    xt_d = nc.dram_tensor("xt", [P, NT, CC, 512], f16, kind="ExternalInput").ap()
    ctxt_d = nc.dram_tensor("ctxt", [P, CC, MP], f16, kind="ExternalInput").ap()
    wqt_d = nc.dram_tensor("wqt", [P, CC, VD], f16, kind="ExternalInput").ap()
    wkt_d = nc.dram_tensor("wkt", [P, CC, VD], f16, kind="ExternalInput").ap()
    wvt_d = nc.dram_tensor("wvt", [P, CC, VD], f16, kind="ExternalInput").ap()
    wot_d = nc.dram_tensor("wot", [P, QT, C], f16, kind="ExternalInput").ap()
    f16c_d = nc.dram_tensor("f16c", [P, W16], f16, kind="ExternalInput").ap()
    f32c_d = nc.dram_tensor("f32c", [P, W32], f32, kind="ExternalInput").ap()
    y_d = nc.dram_tensor("y", [N, C], f16, kind="ExternalOutput").ap()

    with tile.TileContext(nc) as tc:
        with (
            tc.tile_pool(name="consts", bufs=1) as consts,
            tc.tile_pool(name="xpool", bufs=1) as xpool,
            tc.tile_pool(name="kv", bufs=1) as kvp,
            tc.tile_pool(name="work", bufs=2) as work,
            tc.tile_pool(name="ptp", bufs=8) as ptp,
            tc.tile_pool(name="outp", bufs=1) as outp,
            # PSUM budget (8 banks): "big" = [128,1024] 2-bank tiles,
            # bufs=2 -> 4 banks (scores pairs, KV proj).  "acc" =
            # [128,512] 1-bank, bufs=3 -> 3 banks (pv01, pv23, den).
            # "mi" = [128,512] 1-bank, bufs=1 (Qproj / ss / qbc / bcn).
            tc.tile_pool(name="big", bufs=2, space="PSUM") as bigp,
            tc.tile_pool(name="acc", bufs=3, space="PSUM") as accp,
            tc.tile_pool(name="mi", bufs=1, space="PSUM") as mip,
        ):
            # ---- input DMA: batched triggers, split across the two
            # hwdge queues (sync + ACT) so descriptor-gen parallelizes.
            f32c_sb = consts.tile([P, W32], f32)
            nc.sync.dma_start(f32c_sb[:], f32c_d[:])
            f16c_sb = consts.tile([P, W16], f16)
            nc.sync.dma_start(f16c_sb[:], f16c_d[:])
            ctx_sb = xpool.tile([P, CC, MP], f16)
            h1 = CC // 2
            nc.sync.dma_start(ctx_sb[:, 0:h1, :], ctxt_d[:, 0:h1, :])
            nc.sync.dma_start(ctx_sb[:, h1:CC, :], ctxt_d[:, h1:CC, :])
            wk_sb = consts.tile([P, CC, VD], f16)
            nc.sync.dma_start(wk_sb[:], wkt_d[:])
            wv_sb = consts.tile([P, CC, VD], f16)
            nc.sync.dma_start(wv_sb[:], wvt_d[:])
            wq_sb = consts.tile([P, CC, VD], f16)
            nc.sync.dma_start(wq_sb[:], wqt_d[:])
            # x blocks + wo follow on the same queue (consumption order;
            # a second parallel queue would steal HBM bandwidth from the
            # critical ctx/wk path)
            xt_sb = xpool.tile([P, NT, CC, 512], f16)
            for nt_ in range(NT):
                nc.sync.dma_start(xt_sb[:, nt_, :, :], xt_d[:, nt_, :, :])
            wo_sb = consts.tile([P, QT, C], f16)
            nc.sync.dma_start(wo_sb[:], wot_d[:])

            # views into the const packs
            ind2_sb = f16c_sb[:, W16_IND2 : W16_IND2 + 2]
            gqi_sb = f16c_sb[:, W16_GQI : W16_GQI + P]
            gki_sb = f16c_sb[:, W16_GKI : W16_GKI + P]
            m16_sb = f16c_sb[:, W16_M16 : W16_M16 + MC]
            bq_sb = f32c_sb[:, W32_BQ : W32_BQ + QT]
            bk_sb = f32c_sb[:, W32_BK : W32_BK + QT]
            m32_sb = f32c_sb[:, W32_M32 : W32_M32 + MC]

            def bvbm_sl(mc):
                return f16c_sb[:, W16_BVBM + mc * VD : W16_BVBM + (mc + 1) * VD]

            # 34-col ss stationaries: S0 puts tile-0 sums at rows {0,1},
            # S1 puts tile-1 sums at rows {32,33}; all other rows get an
            # explicit 0 write (accumulated) so the [0:34] Ln read below
            # never touches stale psum rows.
            ind34a_sb = f16c_sb[:, W16_IND34A : W16_IND34A + 34]
            ind34b_sb = f16c_sb[:, W16_IND34B : W16_IND34B + 34]
            z97_sb = f16c_sb[:, W16_Z97 : W16_Z97 + 97]

            eps_sb = consts.tile([P, 1], f32)
            nc.vector.memset(eps_sb[:], EPS)
            ones64_sb = consts.tile([P, 64], f16)
            nc.vector.memset(ones64_sb[:], 1.0)

            # ================= KV phase =================
            # K projection: out [kdim, m] (2 tiles of 128 kdims)
            ktn = [kvp.tile([P, MP], f16, name=f"ktn{t}") for t in range(QT)]
            kraw = [kvp.tile([P, MP], f16, name=f"kraw{t}") for t in range(QT)]
            for t in range(QT):
                ps_k = bigp.tile([P, 1024], f32, tag="big")
                for cc in range(CC):
                    for ms in range(0, MP, 512):
                        me = min(ms + 512, MP)
                        nc.tensor.matmul(
                            ps_k[:, ms:me],
                            wk_sb[:, cc, t * P : (t + 1) * P],
                            ctx_sb[:, cc, ms:me],
                            start=(cc == 0),
                            stop=(cc == CC - 1),
                        )
                nc.vector.tensor_scalar_add(
                    kraw[t][:], ps_k[:, :MP], bk_sb[:, t : t + 1]
                )
                sq = work.tile([P, MP], f16, tag="ksq", name="ksq")
                nc.vector.tensor_mul(sq[:], kraw[t][:], kraw[t][:])
                rsl = slice(32 * t, 32 * t + 2)
                ps_ss = bigp.tile([P, 1024], f32, tag="big", name=f"kss{t}")
                for ms in range(0, MP, 512):
                    me = min(ms + 512, MP)
                    nc.tensor.matmul(
                        ps_ss[rsl, ms:me],
                        ind2_sb[:],
                        sq[:, ms:me],
                        start=True,
                        stop=True,
                    )
                # rsqrt(mean_sq + eps) = Exp(-0.5 * Ln(ss/D + eps)); Ln and
                # Exp share one ACT table set so no table switches ever.
                srt = work.tile([34, MP], f32, tag="ksrt", name="ksrt", bufs=1)
                nc.scalar.activation(
                    srt[rsl, :], ps_ss[rsl, :MP], AF.Ln, scale=1.0 / D,
                    bias=eps_sb[rsl, :],
                )
                rstd16 = work.tile([34, MP], f16, tag="krstd16", name="krstd16")
                nc.scalar.activation(rstd16[rsl, :], srt[rsl, :], AF.Exp, scale=-0.5)
                ps_bc = bigp.tile([P, 1024], f32, tag="big", name=f"kbc{t}")
                for ms in range(0, MP, 512):
                    me = min(ms + 512, MP)
                    nc.tensor.matmul(
                        ps_bc[:, ms:me],
                        gki_sb[rsl, :],
                        rstd16[rsl, ms:me],
                        start=True,
                        stop=True,
                    )
                nc.vector.tensor_mul(ktn[t][:], kraw[t][:], ps_bc[:, :MP])

            # V projection directly in [m, vdim] layout + bias + mask
            vt = []
            for mc in range(MC):
                pool = mip if mc % 2 == 0 else accp
                ps_v = pool.tile(
                    [P, 512], f32, tag=("mi" if mc % 2 == 0 else "acc"),
                    name=f"v{mc}",
                )
                for cc in range(CC):
                    nc.tensor.matmul(
                        ps_v[:, 0:VD],
                        ctx_sb[:, cc, mc * P : (mc + 1) * P],
                        wv_sb[:, cc, :],
                        start=(cc == 0),
                        stop=(cc == CC - 1),
                    )
                vtile = kvp.tile([P, VD], f16, name=f"vt{mc}")
                # v = vproj * maskcol + (bv * maskcol)
                nc.vector.scalar_tensor_tensor(
                    out=vtile[:],
                    in0=ps_v[:, 0:VD],
                    scalar=m32_sb[:, mc : mc + 1],
                    in1=bvbm_sl(mc),
                    op0=MUL,
                    op1=mybir.AluOpType.add,
                )
                vt.append(vtile)

            # ================= main pipelined loop =================
            # Per iteration `step`:
            #   - attention of block ant = step-1 (scores/exp/pv/den)
            #   - Q-proj + rms + qtn finalize of block `step`
            #   - normalize (bcn broadcast + outtn stt) of block step-2,
            #     issued after scores(mc2) so the PE queue never waits
            #     on the DVE recip chain at a block boundary.
            outtn = [
                outp.tile([P, N], f16, name=f"outtn{t}") for t in range(QT)
            ]

            # slot schedule within an iteration (keys: mc slot index)
            QW_SLOT = {1: [0, 1], 2: [2, 3], 3: [4]}
            PV_SLOT = {3: [0, 1], 4: [2], 5: [3, 4]}
            BCN_SLOT = 2
            QBC_SLOT = 4
            if MC < 5:
                # degenerate masks: fall back to simple spread
                QW_SLOT = {i: [i] for i in range(min(MC, 5))}
                for i in range(min(MC, 5), 5):
                    QW_SLOT.setdefault(MC, []).append(i)
                PV_SLOT = {MC: list(range(MC))}
                BCN_SLOT = min(2, MC)
                QBC_SLOT = min(4, MC)

            qstate = [None]   # (raw16, sq16, qtn) of block `step`
            yh_state = [0, None]  # [next half index, current y_sb tile]

            def y_half_unit(on_act=False):
                """One out-proj half [128,512] through the mi chain."""
                h = yh_state[0]
                yh_state[0] = h + 1
                tcn, half = divmod(h, 2)
                tsl = slice(tcn * P, (tcn + 1) * P)
                ysl = slice(half * 512, (half + 1) * 512)
                ps_y = mip.tile([P, 512], f32, tag="mi", name="ps_y")
                for t in range(QT):
                    nc.tensor.matmul(
                        ps_y[:],
                        outtn[t][:, tsl],
                        wo_sb[:, t, ysl],
                        start=(t == 0),
                        stop=(t == QT - 1),
                    )
                if half == 0:
                    yh_state[1] = work.tile(
                        [P, 1024], f16, tag="ysb", name="ysb", bufs=4
                    )
                y_sb = yh_state[1]
                if on_act:
                    nc.scalar.activation(y_sb[:, ysl], ps_y, AF.Copy)
                else:
                    nc.vector.tensor_copy(y_sb[:, ysl], ps_y)
                if half == 1:
                    nc.sync.dma_start(y_d[tsl, :], y_sb[:])

            r16_state = [None]
            aqtn_state = [None]  # qtn tiles of the block being attended
            # pending normalize: (pr-> ps_pv tiles, ps_den, rd16, ansl)
            norm_state = [None]

            def qwork_slice(step, raw16, sq16, i):
                """Issue the i-th slice of block `step`'s Q-proj/rms."""
                nsl_t = step  # xt_sb block index
                if i in (0, 1, 2, 3):
                    t, piece = divmod(i, 2)
                    if piece == 0:
                        qps[0] = mip.tile([P, 512], f32, tag="mi", name=f"q{t}")
                    ps_q = qps[0]
                    for cc in range(4 * piece, 4 * piece + 4):
                        nc.tensor.matmul(
                            ps_q[:],
                            wq_sb[:, cc, t * P : (t + 1) * P],
                            xt_sb[:, nsl_t, cc, :],
                            start=(cc == 0),
                            stop=(cc == CC - 1),
                        )
                    if piece == 1:
                        nc.vector.tensor_scalar_add(
                            raw16[t][:], ps_q[:], bq_sb[:, t : t + 1]
                        )
                        nc.vector.tensor_mul(
                            sq16[t][:], raw16[t][:], raw16[t][:]
                        )
                elif i == 4:
                    # ss pair: rows 0:2 (tile0) and 32:34 (tile1); middle
                    # rows written 0 so the Ln read of [0:34] is race-free.
                    ps_ss = mip.tile([P, 512], f32, tag="mi", name="qss")
                    nc.tensor.matmul(
                        ps_ss[0:34, :], ind34a_sb, sq16[0][:],
                        start=True, stop=False,
                    )
                    nc.tensor.matmul(
                        ps_ss[0:34, :], ind34b_sb, sq16[1][:],
                        start=False, stop=True,
                    )
                    srt = work.tile([34, 512], f32, tag="qsrt", name="qsrt")
                    nc.scalar.activation(
                        srt[:], ps_ss[0:34, :], AF.Ln, scale=1.0 / D,
                        bias=eps_sb[0:34, :],
                    )
                    r16 = work.tile([34, 512], f16, tag="qr16", name="qr16")
                    nc.scalar.activation(r16[:], srt[:], AF.Exp, scale=-0.5)
                    r16_state[0] = r16

            def issue_norm():
                """bcn broadcast + outtn normalize of the pending block.

                The per-(head, query) reciprocal rows {0,32,64,96} of rd16
                are broadcast across each head's 64 v-dim partitions on the
                (otherwise idle) GPSIMD engine -- no PE or PSUM involved,
                so the PE queue never waits on this chain.
                """
                if norm_state[0] is None:
                    return
                ps_pv_p, ps_den_p, rd16_p, ansl_p = norm_state[0]
                norm_state[0] = None
                for pr in range(2):
                    ps_bcn = mip.tile([P, 512], f32, tag="mi", name=f"bcn{pr}")
                    for hh in range(2):
                        h = 2 * pr + hh
                        nc.tensor.matmul(
                            ps_bcn[64 * hh : 64 * hh + 64, :],
                            ones64_sb[32 * h : 32 * h + 1, :],
                            rd16_p[32 * h : 32 * h + 1, :],
                            start=True,
                            stop=True,
                            tile_position=(32 * h, 64 * hh),
                            skip_group_check=True,
                        )
                    bcn_sb = work.tile(
                        [P, 512], f16, tag=f"bcn{pr}", name=f"bcn{pr}"
                    )
                    nc.vector.tensor_copy(bcn_sb[:], ps_bcn[:])
                    # ps_den holds den/256 (mask stationary is 1/256); the
                    # stt scalar 1/256 compensates exactly.
                    nc.vector.scalar_tensor_tensor(
                        out=outtn[pr][:, ansl_p],
                        in0=ps_pv_p[pr][:],
                        scalar=1.0 / 256.0,
                        in1=bcn_sb[:],
                        op0=MUL,
                        op1=MUL,
                    )

            for step in range(NT + 1):
                do_q = step < NT
                do_attn = step > 0
                ant = step - 1  # attention block index

                if do_q:
                    raw16 = [
                        work.tile([P, 512], f16, tag=f"qraw{t}", name=f"qraw{t}")
                        for t in range(QT)
                    ]
                    sq16 = [
                        work.tile([P, 512], f16, tag=f"qsq{t}", name=f"qsq{t}")
                        for t in range(QT)
                    ]
                    qtn_tiles = [
                        work.tile([P, 512], f16, tag=f"qtn{t}", name=f"qtn{t}")
                        for t in range(QT)
                    ]
                    qstate[0] = (raw16, sq16, qtn_tiles)
                else:
                    raw16 = sq16 = qtn_tiles = None

                qps = [None]
                aqtn = aqtn_state[0]

                if do_attn:
                    ansl = slice(ant * 512, (ant + 1) * 512)
                    pt_tiles = {}
                    ps_pv = None
                    ps_den = None

                n_slots = (MC + 1) if do_attn else (MC + 1)
                qi_sched = QW_SLOT if do_q else {}

                for mc in range(n_slots):
                    # scores for both head pairs, row-tiled (K=64)
                    if do_attn and mc < MC:
                        pt_pair = []
                        for pr in range(2):
                            ps_s = bigp.tile(
                                [P, 1024], f32, tag="big", name=f"s{mc}_{pr}"
                            )
                            kt = ktn[pr]
                            qt = aqtn[pr]
                            msl = slice(mc * P, (mc + 1) * P)
                            nc.tensor.matmul(
                                ps_s[:, 0:512], kt[0:64, msl], qt[0:64, :],
                                start=True, stop=True,
                            )
                            nc.tensor.matmul(
                                ps_s[:, 512:1024], kt[64:128, msl], qt[64:128, :],
                                start=True, stop=True,
                            )
                            pt = ptp.tile([P, 1024], f16, tag="pt")
                            nc.scalar.activation(pt[:], ps_s[:], AF.Exp)
                            pt_pair.append(pt)
                        pt_tiles[mc] = pt_pair

                    # deferred normalize of block ant-1 (or step-2)
                    if mc == BCN_SLOT:
                        issue_norm()

                    # Q-proj slices of block `step`
                    if do_q:
                        for i in qi_sched.get(mc, []):
                            qwork_slice(step, raw16, sq16, i)

                    # qtn finalize of block `step` (uses this step's r16).
                    # t=0 before the pv group, t=1 after -- the pv matmuls
                    # between them cover the DVE read of the shared mi bank
                    # so the PE queue doesn't micro-stall.
                    def qbc(t):
                        r16_cur = r16_state[0]
                        ps_bc = mip.tile([P, 512], f32, tag="mi", name=f"qbc{t}")
                        nc.tensor.matmul(
                            ps_bc[:],
                            gqi_sb[32 * t : 32 * t + 2, :],
                            r16_cur[32 * t : 32 * t + 2, :],
                            start=True,
                            stop=True,
                        )
                        nc.vector.tensor_mul(
                            qtn_tiles[t][:], raw16[t][:], ps_bc[:]
                        )

                    if do_q and mc == QBC_SLOT:
                        qbc(0)

                    # pv/den groups (lag-3 behind scores)
                    if do_attn:
                        for pmc in PV_SLOT.get(mc, []):
                            if pmc >= MC:
                                continue
                            if ps_pv is None:
                                ps_pv = [
                                    accp.tile(
                                        [P, 512], f32, tag="acc", name=f"pv{pr}"
                                    )
                                    for pr in range(2)
                                ]
                                ps_den = accp.tile(
                                    [P, 512], f32, tag="acc", name="den"
                                )
                                # zero-fill rows 0:97 so the [0:97] recip
                                # read below never touches stale psum.
                                nc.tensor.matmul(
                                    ps_den[0:97, :],
                                    z97_sb,
                                    ktn[0][0:128, 0:512],
                                    start=True,
                                    stop=False,
                                    skip_group_check=True,
                                )
                            pt_pair = pt_tiles[pmc]
                            for pr in range(2):
                                pt = pt_pair[pr]
                                for hh in range(2):
                                    h = 2 * pr + hh
                                    nc.tensor.matmul(
                                        ps_pv[pr][64 * hh : 64 * hh + 64, :],
                                        vt[pmc][:, 64 * h : 64 * h + 64],
                                        pt[:, 512 * hh : 512 * hh + 512],
                                        start=(pmc == 0),
                                        stop=(pmc == MC - 1),
                                        tile_position=(0, 64 * hh),
                                        skip_group_check=True,
                                    )
                            for pr in range(2):
                                pt = pt_pair[pr]
                                for hh in range(2):
                                    h = 2 * pr + hh
                                    nc.tensor.matmul(
                                        ps_den[32 * h : 32 * h + 1, :],
                                        m16_sb[:, pmc : pmc + 1],
                                        pt[:, 512 * hh : 512 * hh + 512],
                                        start=False,
                                        stop=(pmc == MC - 1),
                                        tile_position=(0, 32 * h),
                                        skip_group_check=True,
                                    )

                    if do_q and mc == QBC_SLOT:
                        qbc(1)

                    # out-proj half-units for completed query blocks ride the
                    # loop's engine slack (PE +0.9us/blk, DVE copies, sync DMA).
                    # Only blocks whose normalize finished LAST iter (< ant-1)
                    # are eligible, so the PE queue never waits on an stt.
                    # (disabled: measured -- mi-chain stalls cost more than
                    # the tail saves)
                    if False and do_attn and mc in (1, MC):
                        y_half_unit()

                # end of iteration: recip of this block's den; normalize
                # itself is deferred into the next iteration (BCN_SLOT).
                if do_attn:
                    rd32 = work.tile([P, 512], f32, tag="rd32", name="rd32")
                    nc.vector.reciprocal_approx_fast(
                        out=rd32[0:97, :], in_=ps_den[0:97, :]
                    )
                    rd16 = work.tile([P, 512], f16, tag="rd16", name="rd16")
                    nc.vector.tensor_copy(rd16[0:97, :], rd32[0:97, :])
                    norm_state[0] = (ps_pv, ps_den, rd16, ansl)

                if do_q:
                    aqtn_state[0] = qtn_tiles

            # normalize of the final block
            issue_norm()

            # ================= output projection =================
            # 64 half-units [128,512] rotating through 8 psum banks:
            # groups of 8 halves = [bigA.lo, bigA.hi, acc0, acc1, acc2,
            # mi, bigB.lo, bigB.hi].  Copies alternate ACT/DVE; one
            # merged [128,1024] DMA per 128-row tile (32 triggers).
            n_tiles = N // P  # 32

            def slot_cycle():
                """Infinite cycle of tail psum slots: ('big', tile-or-None)
                pairs where big tiles carry 2 halves with one wide copy."""
                while True:
                    bigA = bigp.tile([P, 1024], f32, tag="big", name="ytA")
                    yield ("bigpair", bigA)
                    for a in range(3):
                        acc_t = accp.tile(
                            [P, 512], f32, tag="acc", name=f"yacc{a}"
                        )
                        yield ("single", acc_t)
                    mi_t = mip.tile([P, 512], f32, tag="mi", name="ymi")
                    yield ("single", mi_t)
                    bigB = bigp.tile([P, 1024], f32, tag="big", name="ytB")
                    yield ("bigpair", bigB)

            slots = slot_cycle()
            h = yh_state[0]
            assert h % 2 == 0
            y_sb_cur = [None]
            ns = 0  # single-slot counter for copy-engine balance
            while h < 2 * n_tiles:
                kind, tile_ = next(slots)
                if kind == "bigpair" and h + 1 < 2 * n_tiles and h % 2 == 0:
                    tcn = h // 2
                    tsl = slice(tcn * P, (tcn + 1) * P)
                    for half in range(2):
                        ysl = slice(half * 512, (half + 1) * 512)
                        for t in range(QT):
                            nc.tensor.matmul(
                                tile_[:, ysl],
                                outtn[t][:, tsl],
                                wo_sb[:, t, ysl],
                                start=(t == 0),
                                stop=(t == QT - 1),
                            )
                    y_sb = work.tile(
                        [P, 1024], f16, tag="ysb", name="ysb", bufs=4
                    )
                    # one wide ACT copy for both halves
                    nc.scalar.activation(y_sb[:], tile_[:], AF.Copy)
                    eng = nc.sync if (tcn % 2 == 0) else nc.scalar
                    eng.dma_start(y_d[tsl, :], y_sb[:])
                    h += 2
                else:
                    if kind == "bigpair":
                        tile_ = tile_[:, 0:512]
                    else:
                        tile_ = tile_[:]
                    tcn, half = divmod(h, 2)
                    tsl = slice(tcn * P, (tcn + 1) * P)
                    ysl = slice(half * 512, (half + 1) * 512)
                    for t in range(QT):
                        nc.tensor.matmul(
                            tile_,
                            outtn[t][:, tsl],
                            wo_sb[:, t, ysl],
                            start=(t == 0),
                            stop=(t == QT - 1),
                        )
                    if half == 0:
                        y_sb_cur[0] = work.tile(
                            [P, 1024], f16, tag="ysb", name="ysb", bufs=4
                        )
                    y_sb = y_sb_cur[0]
                    nc.vector.tensor_copy(y_sb[:, ysl], tile_)
                    ns += 1
                    if half == 1:
                        eng = nc.sync if (tcn % 2 == 0) else nc.scalar
                        eng.dma_start(y_d[tsl, :], y_sb[:])
                    h += 1

    nc.compile()
    _CACHE[key] = nc
    return nc


def _prep(x, context, context_mask, Wq, bq, Wkv, bkv, gq, gk, Wo, bo):
    """Host-side: compaction, transposes, per-core weight slices."""
    f16 = np.float16
    f32 = np.float32
    mask = np.asarray(context_mask)
    idxs = [np.nonzero(mask[b])[0] for b in range(B)]
    mv = [len(ix) for ix in idxs]
    MC = max(1, (max(mv) + P - 1) // P)
    MP = MC * P

    # compacted, padded, transposed context per batch (fp16)
    ctxt = []
    for b in range(B):
        cc = np.zeros((MP, C), dtype=f32)
        cc[: mv[b]] = np.asarray(context[b], dtype=f32)[idxs[b]]
        ctxt.append(np.ascontiguousarray(cc.T, dtype=f16))

    # mask columns [128, MC] per batch
    m32 = []
    for b in range(B):
        m = np.zeros((MP,), dtype=f32)
        m[: mv[b]] = 1.0
        m32.append(np.ascontiguousarray(m.reshape(MC, P).T))

    def perm(a, p):
        """[(o p), m...] -> [p, o, m...] contiguous, fp16."""
        a = np.asarray(a, dtype=f32)
        o = a.shape[0] // p
        return np.ascontiguousarray(
            a.reshape(o, p, *a.shape[1:]).swapaxes(0, 1), dtype=f16
        )

    xt = []
    for b in range(B):
        xb = perm(np.asarray(x[b], dtype=f32).T, P)  # [P, CC, N]
        xt.append(
            np.ascontiguousarray(
                xb.reshape(P, CC, NT, 512).transpose(0, 2, 1, 3)
            )
        )  # [P, NT, CC, 512]
    ctxt = [perm(cc, P) for cc in ctxt]

    ind2 = np.zeros((P, 2), dtype=f16)
    ind2[0:64, 0] = 1.0
    ind2[64:128, 1] = 1.0
    ind34 = np.zeros((P, 68), dtype=f16)
    ind34[0:64, 0] = 1.0
    ind34[64:128, 1] = 1.0
    ind34[0:64, 34 + 32] = 1.0
    ind34[64:128, 34 + 33] = 1.0

    Wq = np.asarray(Wq, dtype=f32)
    Wkv = np.asarray(Wkv, dtype=f32)
    Wo = np.asarray(Wo, dtype=f32)
    bq = np.asarray(bq, dtype=f32)
    bkv = np.asarray(bkv, dtype=f32)
    gq = np.asarray(gq, dtype=f32)
    gk = np.asarray(gk, dtype=f32)

    in_maps = []
    for c in range(8):
        bi, hg = c // 4, c % 4
        hs = slice(VD * hg, VD * (hg + 1))  # 256 dims for 4 heads
        heads = [hg * HC + j for j in range(HC)]

        gqi = np.zeros((P, P), dtype=f16)
        gki = np.zeros((P, P), dtype=f16)
        for t in range(QT):
            for j in range(2):
                h = heads[2 * t + j]
                gqi[32 * t + j, 64 * j : 64 * j + 64] = (
                    gq[h] * (1.0 / np.sqrt(D))
                ).astype(f16)
                gki[32 * t + j, 64 * j : 64 * j + 64] = gk[h].astype(f16)

        bv = bkv[C + VD * hg : C + VD * (hg + 1)]
        bvbm = np.zeros((P, MC, VD), dtype=f16)
        for mc in range(MC):
            bvbm[:, mc, :] = (
                m32[bi][:, mc : mc + 1] * bv[None, :]
            ).astype(f16)

        # packed const tensors (offsets must match device-side slots)
        W16_BVBM = 64 + 128 + 128 + 64 + 64 + 64 + 128
        f16c = np.zeros((P, W16_BVBM + MC * VD), dtype=f16)
        f16c[:, 0:2] = ind2
        f16c[:, 64 : 64 + 128] = gqi
        f16c[:, 192 : 192 + 128] = gki
        f16c[:, 320 : 320 + MC] = (m32[bi] / 256.0).astype(f16)
        f16c[:, 384 : 384 + 34] = ind34[:, 0:34]
        f16c[:, 448 : 448 + 34] = ind34[:, 34:68]
        # z97 slot at 512 stays zero
        f16c[:, W16_BVBM :] = bvbm.reshape(P, MC * VD)
        f32c = np.zeros((P, 192), dtype=f32)
        f32c[:, 0:QT] = np.ascontiguousarray(bq[hs].reshape(QT, P).T)
        f32c[:, 64 : 64 + QT] = np.ascontiguousarray(bkv[hs].reshape(QT, P).T)
        f32c[:, 128 : 128 + MC] = m32[bi]

        in_maps.append(
            {
                "xt": xt[bi],
                "ctxt": ctxt[bi],
                "wqt": perm(Wq[hs].T, P),
                "wkt": perm(Wkv[hs].T, P),
                "wvt": perm(Wkv[C + VD * hg : C + VD * (hg + 1)].T, P),
                "wot": perm(Wo[:, hs].T, P),
                "f16c": np.ascontiguousarray(f16c),
                "f32c": np.ascontiguousarray(f32c),
            }
        )
    return in_maps, MC


def _run(in_maps, MC, **spmd_kwargs):
    from concourse import bass_utils

    nc = _build(MC)
    return bass_utils.run_bass_kernel_spmd(
        nc, in_maps, core_ids=list(range(8)), **spmd_kwargs
    )


def kernel(x, context, context_mask, Wq, bq, Wkv, bkv, gq, gk, Wo, bo):
    in_maps, MC = _prep(
        x, context, context_mask, Wq, bq, Wkv, bkv, gq, gk, Wo, bo
    )
    res = _run(in_maps, MC)
    y = np.zeros((B, N, C), dtype=np.float64)
    for c in range(8):
        y[c // 4] += res.results[c]["y"].astype(np.float64)
    y += np.asarray(bo, dtype=np.float64)[None, None, :]
    return y.astype(np.float32)


# revision 21
# speedup vs baseline: 1.0660x; 1.0303x over previous
"""Cross-attention kernel for Trainium2, sharded over 8 NeuronCores.

Problem (hardcoded shapes): B=2, N=4096, M=1024, DIM=1024, H=16, D=64.
  q = rms_norm(x @ Wq.T + bq)        per-head, gamma gq, eps 1e-6
  k = rms_norm(ctx @ Wk.T + bk)      (Wk = first half of Wkv)
  v = ctx @ Wv.T + bv                (Wv = second half of Wkv)
  out = softmax(q k^T / sqrt(D) + mask_bias) @ v
  y = out @ Wo.T + bo
Sharding: 2 batches x 4 head-groups -> 8 cores.  Core c handles batch
c//4 and heads [4*(c%4), 4*(c%4)+4).  Host sums the 4 partial y's per
batch (row-sharded Wo) and adds bo.

Device-side design notes (v2):
 - Context compacted on host to valid tokens, padded to MC*128.
 - All activations/weights fp16 (fp32 PSUM).  exp(s) bounded by e^8.
 - Startup: small consts packed into 2 host tensors (2 DMA triggers
   instead of 7); x pre-tiled [P, NT, CC, 512] so each block slice is
   one contiguous descriptor set; x/wo triggers issued from the ACT
   queue (hwdge) in parallel with the sync queue to beat the ~635ns
   serial descriptor-gen cost per DMA trigger.
 - Main loop: per 512-query block, attention of block X-1 interleaves
   with Q-proj/rms of block X.  The reciprocal-broadcast (bcn) of
   block X-1 is issued mid block X (mi-pool psum, after scores mc2)
   so the PE in-order queue never waits on the DVE recip chain at a
   block boundary -- that stall was re-throttling HAM every block.
 - PV lags scores by 3 chunks so the acc-psum allocs of block X land
   after the normalize reads of block X-1 (no rotation stall).
 - Out-projection tail: 64 half-units [128,512] rotating through 8
   psum banks (2x big as halves + 3 acc + 1 mi), psum->sbuf copies
   alternating ACT/DVE, one merged [128,1024] DMA per 128-row tile
   (32 triggers instead of 64).
"""

import numpy as np

P = 128
B = 2
N = 4096
M = 1024
C = 1024  # DIM == COND_DIM
H = 16
D = 64
HC = 4  # heads per core
VD = HC * D  # 256 v/q/k dims per core
CC = C // P  # contraction chunks (8)
NT = N // 512  # query blocks of 512 (8)
QT = 2  # qdim tiles of 128 (VD / P)
EPS = 1e-6

_CACHE = {}


def _build(MC, dbg=False):
    """Build the kernel for MC context chunks of 128 (M_pad = 128*MC)."""
    key = ("nc", MC, dbg)
    if key in _CACHE:
        return _CACHE[key]

    import concourse.bass as bass  # noqa: F401
    import concourse.tile as tile
    from concourse import bacc, mybir

    f32 = mybir.dt.float32
    f16 = mybir.dt.float16
    AF = mybir.ActivationFunctionType
    MUL = mybir.AluOpType.mult
    MP = MC * P  # padded context length

    nc = bacc.Bacc("TRN2", target_bir_lowering=False, debug=False, num_devices=8)

    # All ACT functions used here (Exp, Ln, Copy, Identity) live in the
    # single table set "natural_log_exp_and_others".  The default set
    # assignment pass picks a different set per function and thrashes
    # ~20 ACT_TABLE_LOADs (~1.3us each); restrict the candidate list so
    # the fixpoint pass hoists ONE load to kernel entry.
    import types as _types
    import bass_rust as _bass_rust
    from concourse.hw_specs import get_activation_tables as _gat

    def _act_loads_single_set(self):
        has_act = any(
            isinstance(i, mybir.InstActivation)
            for b in self.main_func.blocks
            for i in b.instructions
        )
        if not has_act:
            return
        tables = list(_gat(self.m.arch).items())
        keep = "natural_log_exp_and_others"
        filtered = [(n, (set(fns) if n == keep else set())) for n, fns in tables]
        _bass_rust.insert_act_table_loads(self, filtered)

    nc.insert_act_table_loads = _types.MethodType(_act_loads_single_set, nc)

    # packed const widths, each slot 64-element aligned so matmul
    # stationary reads (FWL does 32-bit chunked fetches) stay aligned.
    W16_IND2 = 0
    W16_GQI = 64
    W16_GKI = W16_GQI + P
    W16_M16 = W16_GKI + P
    W16_IND34A = W16_M16 + 64
    W16_IND34B = W16_IND34A + 64
    W16_Z97 = W16_IND34B + 64
    W16_BVBM = W16_Z97 + 128
    W16 = W16_BVBM + MC * VD
    W32_BQ = 0
    W32_BK = 64
    W32_M32 = 128
    W32 = 192

    xt_d = nc.dram_tensor("xt", [P, NT, CC, 512], f16, kind="ExternalInput").ap()
    ctxt_d = nc.dram_tensor("ctxt", [P, CC, MP], f16, kind="ExternalInput").ap()
    wqt_d = nc.dram_tensor("wqt", [P, CC, VD], f16, kind="ExternalInput").ap()
    wkt_d = nc.dram_tensor("wkt", [P, CC, VD], f16, kind="ExternalInput").ap()
    wvt_d = nc.dram_tensor("wvt", [P, CC, VD], f16, kind="ExternalInput").ap()
    wot_d = nc.dram_tensor("wot", [P, QT, C], f16, kind="ExternalInput").ap()
    f16c_d = nc.dram_tensor("f16c", [P, W16], f16, kind="ExternalInput").ap()
    f32c_d = nc.dram_tensor("f32c", [P, W32], f32, kind="ExternalInput").ap()
    y_d = nc.dram_tensor("y", [N, C], f16, kind="ExternalOutput").ap()

    with tile.TileContext(nc) as tc:
        with (
            tc.tile_pool(name="consts", bufs=1) as consts,
            tc.tile_pool(name="xpool", bufs=1) as xpool,
            tc.tile_pool(name="kv", bufs=1) as kvp,
            tc.tile_pool(name="work", bufs=2) as work,
            tc.tile_pool(name="ptp", bufs=8) as ptp,
            tc.tile_pool(name="outp", bufs=1) as outp,
            # PSUM budget (8 banks): "big" = [128,1024] 2-bank tiles,
            # bufs=2 -> 4 banks (scores pairs, KV proj).  "acc" =
            # [128,512] 1-bank, bufs=3 -> 3 banks (pv01, pv23, den).
            # "mi" = [128,512] 1-bank, bufs=1 (Qproj / ss / qbc / bcn).
            tc.tile_pool(name="big", bufs=2, space="PSUM") as bigp,
            tc.tile_pool(name="acc", bufs=3, space="PSUM") as accp,
            tc.tile_pool(name="mi", bufs=1, space="PSUM") as mip,
        ):
            # ---- input DMA: batched triggers, split across the two
            # hwdge queues (sync + ACT) so descriptor-gen parallelizes.
            f32c_sb = consts.tile([P, W32], f32)
            nc.sync.dma_start(f32c_sb[:], f32c_d[:])
            f16c_sb = consts.tile([P, W16], f16)
            nc.sync.dma_start(f16c_sb[:], f16c_d[:])
            ctx_sb = xpool.tile([P, CC, MP], f16)
            h1 = CC // 2
            nc.sync.dma_start(ctx_sb[:, 0:h1, :], ctxt_d[:, 0:h1, :])
            nc.sync.dma_start(ctx_sb[:, h1:CC, :], ctxt_d[:, h1:CC, :])
            wk_sb = consts.tile([P, CC, VD], f16)
            nc.sync.dma_start(wk_sb[:], wkt_d[:])
            wv_sb = consts.tile([P, CC, VD], f16)
            nc.sync.dma_start(wv_sb[:], wvt_d[:])
            wq_sb = consts.tile([P, CC, VD], f16)
            nc.sync.dma_start(wq_sb[:], wqt_d[:])
            # x blocks + wo follow on the same queue (consumption order;
            # a second parallel queue would steal HBM bandwidth from the
            # critical ctx/wk path)
            xt_sb = xpool.tile([P, NT, CC, 512], f16)
            for nt_ in range(NT):
                nc.sync.dma_start(xt_sb[:, nt_, :, :], xt_d[:, nt_, :, :])
            wo_sb = consts.tile([P, QT, C], f16)
            nc.sync.dma_start(wo_sb[:], wot_d[:])

            # views into the const packs
            ind2_sb = f16c_sb[:, W16_IND2 : W16_IND2 + 2]
            gqi_sb = f16c_sb[:, W16_GQI : W16_GQI + P]
            gki_sb = f16c_sb[:, W16_GKI : W16_GKI + P]
            m16_sb = f16c_sb[:, W16_M16 : W16_M16 + MC]
            bq_sb = f32c_sb[:, W32_BQ : W32_BQ + QT]
            bk_sb = f32c_sb[:, W32_BK : W32_BK + QT]
            m32_sb = f32c_sb[:, W32_M32 : W32_M32 + MC]

            def bvbm_sl(mc):
                return f16c_sb[:, W16_BVBM + mc * VD : W16_BVBM + (mc + 1) * VD]

            # 34-col ss stationaries: S0 puts tile-0 sums at rows {0,1},
            # S1 puts tile-1 sums at rows {32,33}; all other rows get an
            # explicit 0 write (accumulated) so the [0:34] Ln read below
            # never touches stale psum rows.
            ind34a_sb = f16c_sb[:, W16_IND34A : W16_IND34A + 34]
            ind34b_sb = f16c_sb[:, W16_IND34B : W16_IND34B + 34]
            z97_sb = f16c_sb[:, W16_Z97 : W16_Z97 + 97]

            eps_sb = consts.tile([P, 1], f32)
            nc.vector.memset(eps_sb[:], EPS)
            ones64_sb = consts.tile([P, 64], f16)
            nc.vector.memset(ones64_sb[:], 1.0)

            # ================= KV phase =================
            # K projection: out [kdim, m] (2 tiles of 128 kdims)
            ktn = [kvp.tile([P, MP], f16, name=f"ktn{t}") for t in range(QT)]
            kraw = [kvp.tile([P, MP], f16, name=f"kraw{t}") for t in range(QT)]
            for t in range(QT):
                ps_k = bigp.tile([P, 1024], f32, tag="big")
                for cc in range(CC):
                    for ms in range(0, MP, 512):
                        me = min(ms + 512, MP)
                        nc.tensor.matmul(
                            ps_k[:, ms:me],
                            wk_sb[:, cc, t * P : (t + 1) * P],
                            ctx_sb[:, cc, ms:me],
                            start=(cc == 0),
                            stop=(cc == CC - 1),
                        )
                nc.vector.tensor_scalar_add(
                    kraw[t][:], ps_k[:, :MP], bk_sb[:, t : t + 1]
                )
                sq = work.tile([P, MP], f16, tag="ksq", name="ksq")
                nc.vector.tensor_mul(sq[:], kraw[t][:], kraw[t][:])
                rsl = slice(32 * t, 32 * t + 2)
                ps_ss = bigp.tile([P, 1024], f32, tag="big", name=f"kss{t}")
                for ms in range(0, MP, 512):
                    me = min(ms + 512, MP)
                    nc.tensor.matmul(
                        ps_ss[rsl, ms:me],
                        ind2_sb[:],
                        sq[:, ms:me],
                        start=True,
                        stop=True,
                    )
                # rsqrt(mean_sq + eps) = Exp(-0.5 * Ln(ss/D + eps)); Ln and
                # Exp share one ACT table set so no table switches ever.
                srt = work.tile([34, MP], f32, tag="ksrt", name="ksrt", bufs=1)
                nc.scalar.activation(
                    srt[rsl, :], ps_ss[rsl, :MP], AF.Ln, scale=1.0 / D,
                    bias=eps_sb[rsl, :],
                )
                rstd16 = work.tile([34, MP], f16, tag="krstd16", name="krstd16")
                nc.scalar.activation(rstd16[rsl, :], srt[rsl, :], AF.Exp, scale=-0.5)
                ps_bc = bigp.tile([P, 1024], f32, tag="big", name=f"kbc{t}")
                for ms in range(0, MP, 512):
                    me = min(ms + 512, MP)
                    nc.tensor.matmul(
                        ps_bc[:, ms:me],
                        gki_sb[rsl, :],
                        rstd16[rsl, ms:me],
                        start=True,
                        stop=True,
                    )
                nc.vector.tensor_mul(ktn[t][:], kraw[t][:], ps_bc[:, :MP])

            # V projection directly in [m, vdim] layout + bias + mask
            vt = []
            for mc in range(MC):
                pool = mip if mc % 2 == 0 else accp
                ps_v = pool.tile(
                    [P, 512], f32, tag=("mi" if mc % 2 == 0 else "acc"),
                    name=f"v{mc}",
                )
                for cc in range(CC):
                    nc.tensor.matmul(
                        ps_v[:, 0:VD],
                        ctx_sb[:, cc, mc * P : (mc + 1) * P],
                        wv_sb[:, cc, :],
                        start=(cc == 0),
                        stop=(cc == CC - 1),
                    )
                vtile = kvp.tile([P, VD], f16, name=f"vt{mc}")
                # v = vproj * maskcol + (bv * maskcol)
                nc.vector.scalar_tensor_tensor(
                    out=vtile[:],
                    in0=ps_v[:, 0:VD],
                    scalar=m32_sb[:, mc : mc + 1],
                    in1=bvbm_sl(mc),
                    op0=MUL,
                    op1=mybir.AluOpType.add,
                )
                vt.append(vtile)

            # ================= main pipelined loop =================
            # Per iteration `step`:
            #   - attention of block ant = step-1 (scores/exp/pv/den)
            #   - Q-proj + rms + qtn finalize of block `step`
            #   - normalize (bcn broadcast + outtn stt) of block step-2,
            #     issued after scores(mc2) so the PE queue never waits
            #     on the DVE recip chain at a block boundary.
            outtn = [
                outp.tile([P, N], f16, name=f"outtn{t}") for t in range(QT)
            ]

            # slot schedule within an iteration (keys: mc slot index)
            QW_SLOT = {1: [0, 1], 2: [2, 3], 3: [4]}
            PV_SLOT = {3: [0, 1], 4: [2], 5: [3, 4]}
            BCN_SLOT = 2
            QBC_SLOT = 4
            if MC < 5:
                # degenerate masks: fall back to simple spread
                QW_SLOT = {i: [i] for i in range(min(MC, 5))}
                for i in range(min(MC, 5), 5):
                    QW_SLOT.setdefault(MC, []).append(i)
                PV_SLOT = {MC: list(range(MC))}
                BCN_SLOT = min(2, MC)
                QBC_SLOT = min(4, MC)

            qstate = [None]   # (raw16, sq16, qtn) of block `step`
            yh_state = [0, None]  # [next half index, current y_sb tile]

            def y_half_unit(on_act=False):
                """One out-proj half [128,512] through the mi chain."""
                h = yh_state[0]
                yh_state[0] = h + 1
                tcn, half = divmod(h, 2)
                tsl = slice(tcn * P, (tcn + 1) * P)
                ysl = slice(half * 512, (half + 1) * 512)
                ps_y = mip.tile([P, 512], f32, tag="mi", name="ps_y")
                for t in range(QT):
                    nc.tensor.matmul(
                        ps_y[:],
                        outtn[t][:, tsl],
                        wo_sb[:, t, ysl],
                        start=(t == 0),
                        stop=(t == QT - 1),
                    )
                if half == 0:
                    yh_state[1] = work.tile(
                        [P, 1024], f16, tag="ysb", name="ysb", bufs=6
                    )
                y_sb = yh_state[1]
                if on_act:
                    nc.scalar.activation(y_sb[:, ysl], ps_y, AF.Copy)
                else:
                    nc.vector.tensor_copy(y_sb[:, ysl], ps_y)
                if half == 1:
                    nc.sync.dma_start(y_d[tsl, :], y_sb[:])

            r16_state = [None]
            aqtn_state = [None]  # qtn tiles of the block being attended
            # pending normalize: (pr-> ps_pv tiles, ps_den, rd16, ansl)
            norm_state = [None]

            def qwork_slice(step, raw16, sq16, i):
                """Issue the i-th slice of block `step`'s Q-proj/rms."""
                nsl_t = step  # xt_sb block index
                if i in (0, 1, 2, 3):
                    t, piece = divmod(i, 2)
                    if piece == 0:
                        qps[0] = mip.tile([P, 512], f32, tag="mi", name=f"q{t}")
                    ps_q = qps[0]
                    for cc in range(4 * piece, 4 * piece + 4):
                        nc.tensor.matmul(
                            ps_q[:],
                            wq_sb[:, cc, t * P : (t + 1) * P],
                            xt_sb[:, nsl_t, cc, :],
                            start=(cc == 0),
                            stop=(cc == CC - 1),
                        )
                    if piece == 1:
                        nc.vector.tensor_scalar_add(
                            raw16[t][:], ps_q[:], bq_sb[:, t : t + 1]
                        )
                        nc.vector.tensor_mul(
                            sq16[t][:], raw16[t][:], raw16[t][:]
                        )
                elif i == 4:
                    # ss pair: rows 0:2 (tile0) and 32:34 (tile1); middle
                    # rows written 0 so the Ln read of [0:34] is race-free.
                    ps_ss = mip.tile([P, 512], f32, tag="mi", name="qss")
                    nc.tensor.matmul(
                        ps_ss[0:34, :], ind34a_sb, sq16[0][:],
                        start=True, stop=False,
                    )
                    nc.tensor.matmul(
                        ps_ss[0:34, :], ind34b_sb, sq16[1][:],
                        start=False, stop=True,
                    )
                    srt = work.tile([34, 512], f32, tag="qsrt", name="qsrt")
                    nc.scalar.activation(
                        srt[:], ps_ss[0:34, :], AF.Ln, scale=1.0 / D,
                        bias=eps_sb[0:34, :],
                    )
                    r16 = work.tile([34, 512], f16, tag="qr16", name="qr16")
                    nc.scalar.activation(r16[:], srt[:], AF.Exp, scale=-0.5)
                    r16_state[0] = r16

            def issue_norm():
                """bcn broadcast + outtn normalize of the pending block.

                The per-(head, query) reciprocal rows {0,32,64,96} of rd16
                are broadcast across each head's 64 v-dim partitions on the
                (otherwise idle) GPSIMD engine -- no PE or PSUM involved,
                so the PE queue never waits on this chain.
                """
                if norm_state[0] is None:
                    return
                ps_pv_p, ps_den_p, rd16_p, ansl_p = norm_state[0]
                norm_state[0] = None
                for pr in range(2):
                    ps_bcn = mip.tile([P, 512], f32, tag="mi", name=f"bcn{pr}")
                    for hh in range(2):
                        h = 2 * pr + hh
                        nc.tensor.matmul(
                            ps_bcn[64 * hh : 64 * hh + 64, :],
                            ones64_sb[32 * h : 32 * h + 1, :],
                            rd16_p[32 * h : 32 * h + 1, :],
                            start=True,
                            stop=True,
                            tile_position=(32 * h, 64 * hh),
                            skip_group_check=True,
                        )
                    bcn_sb = work.tile(
                        [P, 512], f16, tag=f"bcn{pr}", name=f"bcn{pr}"
                    )
                    nc.vector.tensor_copy(bcn_sb[:], ps_bcn[:])
                    # ps_den holds den/256 (mask stationary is 1/256); the
                    # stt scalar 1/256 compensates exactly.
                    nc.vector.scalar_tensor_tensor(
                        out=outtn[pr][:, ansl_p],
                        in0=ps_pv_p[pr][:],
                        scalar=1.0 / 256.0,
                        in1=bcn_sb[:],
                        op0=MUL,
                        op1=MUL,
                    )

            for step in range(NT + 1):
                do_q = step < NT
                do_attn = step > 0
                ant = step - 1  # attention block index

                if do_q:
                    raw16 = [
                        work.tile([P, 512], f16, tag=f"qraw{t}", name=f"qraw{t}")
                        for t in range(QT)
                    ]
                    sq16 = [
                        work.tile([P, 512], f16, tag=f"qsq{t}", name=f"qsq{t}")
                        for t in range(QT)
                    ]
                    qtn_tiles = [
                        work.tile([P, 512], f16, tag=f"qtn{t}", name=f"qtn{t}")
                        for t in range(QT)
                    ]
                    qstate[0] = (raw16, sq16, qtn_tiles)
                else:
                    raw16 = sq16 = qtn_tiles = None

                qps = [None]
                aqtn = aqtn_state[0]

                if do_attn:
                    ansl = slice(ant * 512, (ant + 1) * 512)
                    pt_tiles = {}
                    ps_pv = None
                    ps_den = None

                n_slots = (MC + 1) if do_attn else (MC + 1)
                qi_sched = QW_SLOT if do_q else {}

                for mc in range(n_slots):
                    # scores for both head pairs, row-tiled (K=64)
                    if do_attn and mc < MC:
                        pt_pair = []
                        for pr in range(2):
                            ps_s = bigp.tile(
                                [P, 1024], f32, tag="big", name=f"s{mc}_{pr}"
                            )
                            kt = ktn[pr]
                            qt = aqtn[pr]
                            msl = slice(mc * P, (mc + 1) * P)
                            nc.tensor.matmul(
                                ps_s[:, 0:512], kt[0:64, msl], qt[0:64, :],
                                start=True, stop=True,
                            )
                            nc.tensor.matmul(
                                ps_s[:, 512:1024], kt[64:128, msl], qt[64:128, :],
                                start=True, stop=True,
                            )
                            pt = ptp.tile([P, 1024], f16, tag="pt")
                            nc.scalar.activation(pt[:], ps_s[:], AF.Exp)
                            pt_pair.append(pt)
                        pt_tiles[mc] = pt_pair

                    # deferred normalize of block ant-1 (or step-2)
                    if mc == BCN_SLOT:
                        issue_norm()

                    # Q-proj slices of block `step`
                    if do_q:
                        for i in qi_sched.get(mc, []):
                            qwork_slice(step, raw16, sq16, i)

                    # qtn finalize of block `step` (uses this step's r16).
                    # t=0 before the pv group, t=1 after -- the pv matmuls
                    # between them cover the DVE read of the shared mi bank
                    # so the PE queue doesn't micro-stall.
                    def qbc(t):
                        r16_cur = r16_state[0]
                        ps_bc = mip.tile([P, 512], f32, tag="mi", name=f"qbc{t}")
                        nc.tensor.matmul(
                            ps_bc[:],
                            gqi_sb[32 * t : 32 * t + 2, :],
                            r16_cur[32 * t : 32 * t + 2, :],
                            start=True,
                            stop=True,
                        )
                        nc.vector.tensor_mul(
                            qtn_tiles[t][:], raw16[t][:], ps_bc[:]
                        )

                    if do_q and mc == QBC_SLOT:
                        qbc(0)

                    # pv/den groups (lag-3 behind scores)
                    if do_attn:
                        for pmc in PV_SLOT.get(mc, []):
                            if pmc >= MC:
                                continue
                            if ps_pv is None:
                                ps_pv = [
                                    accp.tile(
                                        [P, 512], f32, tag="acc", name=f"pv{pr}"
                                    )
                                    for pr in range(2)
                                ]
                                ps_den = accp.tile(
                                    [P, 512], f32, tag="acc", name="den"
                                )
                                # zero-fill rows 0:97 so the [0:97] recip
                                # read below never touches stale psum.
                                nc.tensor.matmul(
                                    ps_den[0:97, :],
                                    z97_sb,
                                    ktn[0][0:128, 0:512],
                                    start=True,
                                    stop=False,
                                    skip_group_check=True,
                                )
                            pt_pair = pt_tiles[pmc]
                            for pr in range(2):
                                pt = pt_pair[pr]
                                for hh in range(2):
                                    h = 2 * pr + hh
                                    nc.tensor.matmul(
                                        ps_pv[pr][64 * hh : 64 * hh + 64, :],
                                        vt[pmc][:, 64 * h : 64 * h + 64],
                                        pt[:, 512 * hh : 512 * hh + 512],
                                        start=(pmc == 0),
                                        stop=(pmc == MC - 1),
                                        tile_position=(0, 64 * hh),
                                        skip_group_check=True,
                                    )
                            for pr in range(2):
                                pt = pt_pair[pr]
                                for hh in range(2):
                                    h = 2 * pr + hh
                                    nc.tensor.matmul(
                                        ps_den[32 * h : 32 * h + 1, :],
                                        m16_sb[:, pmc : pmc + 1],
                                        pt[:, 512 * hh : 512 * hh + 512],
                                        start=False,
                                        stop=(pmc == MC - 1),
                                        tile_position=(0, 32 * h),
                                        skip_group_check=True,
                                    )

                    if do_q and mc == QBC_SLOT:
                        qbc(1)

                    # out-proj half-units for completed query blocks ride the
                    # loop's engine slack (PE +0.9us/blk, DVE copies, sync DMA).
                    # Only blocks whose normalize finished LAST iter (< ant-1)
                    # are eligible, so the PE queue never waits on an stt.
                    # (disabled: measured -- mi-chain stalls cost more than
                    # the tail saves)
                    if False and do_attn and mc in (1, MC):
                        y_half_unit()

                # end of iteration: recip of this block's den; normalize
                # itself is deferred into the next iteration (BCN_SLOT).
                if do_attn:
                    rd32 = work.tile([P, 512], f32, tag="rd32", name="rd32")
                    nc.vector.reciprocal_approx_fast(
                        out=rd32[0:97, :], in_=ps_den[0:97, :]
                    )
                    rd16 = work.tile([P, 512], f16, tag="rd16", name="rd16")
                    nc.vector.tensor_copy(rd16[0:97, :], rd32[0:97, :])
                    norm_state[0] = (ps_pv, ps_den, rd16, ansl)

                if do_q:
                    aqtn_state[0] = qtn_tiles

            # normalize of the final block
            issue_norm()

            # ================= output projection =================
            # 64 half-units [128,512] rotating through 8 psum banks:
            # groups of 8 halves = [bigA.lo, bigA.hi, acc0, acc1, acc2,
            # mi, bigB.lo, bigB.hi].  Copies alternate ACT/DVE; one
            # merged [128,1024] DMA per 128-row tile (32 triggers).
            n_tiles = N // P  # 32

            def slot_cycle():
                """Infinite cycle of tail psum slots: ('big', tile-or-None)
                pairs where big tiles carry 2 halves with one wide copy."""
                while True:
                    bigA = bigp.tile([P, 1024], f32, tag="big", name="ytA")
                    yield ("bigpair", bigA)
                    for a in range(3):
                        acc_t = accp.tile(
                            [P, 512], f32, tag="acc", name=f"yacc{a}"
                        )
                        yield ("single", acc_t)
                    mi_t = mip.tile([P, 512], f32, tag="mi", name="ymi")
                    yield ("single", mi_t)
                    bigB = bigp.tile([P, 1024], f32, tag="big", name="ytB")
                    yield ("bigpair", bigB)

            slots = slot_cycle()
            h = yh_state[0]
            assert h % 2 == 0
            y_sb_cur = [None]
            ns = 0  # single-slot counter for copy-engine balance
            while h < 2 * n_tiles:
                kind, tile_ = next(slots)
                if kind == "bigpair" and h + 1 < 2 * n_tiles and h % 2 == 0:
                    tcn = h // 2
                    tsl = slice(tcn * P, (tcn + 1) * P)
                    for half in range(2):
                        ysl = slice(half * 512, (half + 1) * 512)
                        for t in range(QT):
                            nc.tensor.matmul(
                                tile_[:, ysl],
                                outtn[t][:, tsl],
                                wo_sb[:, t, ysl],
                                start=(t == 0),
                                stop=(t == QT - 1),
                            )
                    y_sb = work.tile(
                        [P, 1024], f16, tag="ysb", name="ysb", bufs=6
                    )
                    # one wide ACT copy for both halves
                    nc.scalar.activation(y_sb[:], tile_[:], AF.Copy)
                    eng = nc.sync if (tcn % 2 == 0) else nc.scalar
                    eng.dma_start(y_d[tsl, :], y_sb[:])
                    h += 2
                else:
                    if kind == "bigpair":
                        tile_ = tile_[:, 0:512]
                    else:
                        tile_ = tile_[:]
                    tcn, half = divmod(h, 2)
                    tsl = slice(tcn * P, (tcn + 1) * P)
                    ysl = slice(half * 512, (half + 1) * 512)
                    for t in range(QT):
                        nc.tensor.matmul(
                            tile_,
                            outtn[t][:, tsl],
                            wo_sb[:, t, ysl],
                            start=(t == 0),
                            stop=(t == QT - 1),
                        )
                    if half == 0:
                        y_sb_cur[0] = work.tile(
                            [P, 1024], f16, tag="ysb", name="ysb", bufs=6
                        )
                    y_sb = y_sb_cur[0]
                    nc.vector.tensor_copy(y_sb[:, ysl], tile_)
                    ns += 1
                    if half == 1:
                        eng = nc.sync if (tcn % 2 == 0) else nc.scalar
                        eng.dma_start(y_d[tsl, :], y_sb[:])
                    h += 1

    nc.compile()
    _CACHE[key] = nc
    return nc


def _prep(x, context, context_mask, Wq, bq, Wkv, bkv, gq, gk, Wo, bo):
    """Host-side: compaction, transposes, per-core weight slices."""
    f16 = np.float16
    f32 = np.float32
    mask = np.asarray(context_mask)
    idxs = [np.nonzero(mask[b])[0] for b in range(B)]
    mv = [len(ix) for ix in idxs]
    MC = max(1, (max(mv) + P - 1) // P)
    MP = MC * P

    # compacted, padded, transposed context per batch (fp16)
    ctxt = []
    for b in range(B):
        cc = np.zeros((MP, C), dtype=f32)
        cc[: mv[b]] = np.asarray(context[b], dtype=f32)[idxs[b]]
        ctxt.append(np.ascontiguousarray(cc.T, dtype=f16))

    # mask columns [128, MC] per batch
    m32 = []
    for b in range(B):
        m = np.zeros((MP,), dtype=f32)
        m[: mv[b]] = 1.0
        m32.append(np.ascontiguousarray(m.reshape(MC, P).T))

    def perm(a, p):
        """[(o p), m...] -> [p, o, m...] contiguous, fp16."""
        a = np.asarray(a, dtype=f32)
        o = a.shape[0] // p
        return np.ascontiguousarray(
            a.reshape(o, p, *a.shape[1:]).swapaxes(0, 1), dtype=f16
        )

    xt = []
    for b in range(B):
        xb = perm(np.asarray(x[b], dtype=f32).T, P)  # [P, CC, N]
        xt.append(
            np.ascontiguousarray(
                xb.reshape(P, CC, NT, 512).transpose(0, 2, 1, 3)
            )
        )  # [P, NT, CC, 512]
    ctxt = [perm(cc, P) for cc in ctxt]

    ind2 = np.zeros((P, 2), dtype=f16)
    ind2[0:64, 0] = 1.0
    ind2[64:128, 1] = 1.0
    ind34 = np.zeros((P, 68), dtype=f16)
    ind34[0:64, 0] = 1.0
    ind34[64:128, 1] = 1.0
    ind34[0:64, 34 + 32] = 1.0
    ind34[64:128, 34 + 33] = 1.0

    Wq = np.asarray(Wq, dtype=f32)
    Wkv = np.asarray(Wkv, dtype=f32)
    Wo = np.asarray(Wo, dtype=f32)
    bq = np.asarray(bq, dtype=f32)
    bkv = np.asarray(bkv, dtype=f32)
    gq = np.asarray(gq, dtype=f32)
    gk = np.asarray(gk, dtype=f32)

    in_maps = []
    for c in range(8):
        bi, hg = c // 4, c % 4
        hs = slice(VD * hg, VD * (hg + 1))  # 256 dims for 4 heads
        heads = [hg * HC + j for j in range(HC)]

        gqi = np.zeros((P, P), dtype=f16)
        gki = np.zeros((P, P), dtype=f16)
        for t in range(QT):
            for j in range(2):
                h = heads[2 * t + j]
                gqi[32 * t + j, 64 * j : 64 * j + 64] = (
                    gq[h] * (1.0 / np.sqrt(D))
                ).astype(f16)
                gki[32 * t + j, 64 * j : 64 * j + 64] = gk[h].astype(f16)

        bv = bkv[C + VD * hg : C + VD * (hg + 1)]
        bvbm = np.zeros((P, MC, VD), dtype=f16)
        for mc in range(MC):
            bvbm[:, mc, :] = (
                m32[bi][:, mc : mc + 1] * bv[None, :]
            ).astype(f16)

        # packed const tensors (offsets must match device-side slots)
        W16_BVBM = 64 + 128 + 128 + 64 + 64 + 64 + 128
        f16c = np.zeros((P, W16_BVBM + MC * VD), dtype=f16)
        f16c[:, 0:2] = ind2
        f16c[:, 64 : 64 + 128] = gqi
        f16c[:, 192 : 192 + 128] = gki
        f16c[:, 320 : 320 + MC] = (m32[bi] / 256.0).astype(f16)
        f16c[:, 384 : 384 + 34] = ind34[:, 0:34]
        f16c[:, 448 : 448 + 34] = ind34[:, 34:68]
        # z97 slot at 512 stays zero
        f16c[:, W16_BVBM :] = bvbm.reshape(P, MC * VD)
        f32c = np.zeros((P, 192), dtype=f32)
        f32c[:, 0:QT] = np.ascontiguousarray(bq[hs].reshape(QT, P).T)
        f32c[:, 64 : 64 + QT] = np.ascontiguousarray(bkv[hs].reshape(QT, P).T)
        f32c[:, 128 : 128 + MC] = m32[bi]

        in_maps.append(
            {
                "xt": xt[bi],
                "ctxt": ctxt[bi],
                "wqt": perm(Wq[hs].T, P),
                "wkt": perm(Wkv[hs].T, P),
                "wvt": perm(Wkv[C + VD * hg : C + VD * (hg + 1)].T, P),
                "wot": perm(Wo[:, hs].T, P),
                "f16c": np.ascontiguousarray(f16c),
                "f32c": np.ascontiguousarray(f32c),
            }
        )
    return in_maps, MC


def _run(in_maps, MC, **spmd_kwargs):
    from concourse import bass_utils

    nc = _build(MC)
    return bass_utils.run_bass_kernel_spmd(
        nc, in_maps, core_ids=list(range(8)), **spmd_kwargs
    )


def kernel(x, context, context_mask, Wq, bq, Wkv, bkv, gq, gk, Wo, bo):
    in_maps, MC = _prep(
        x, context, context_mask, Wq, bq, Wkv, bkv, gq, gk, Wo, bo
    )
    res = _run(in_maps, MC)
    y = np.zeros((B, N, C), dtype=np.float64)
    for c in range(8):
        y[c // 4] += res.results[c]["y"].astype(np.float64)
    y += np.asarray(bo, dtype=np.float64)[None, None, :]
    return y.astype(np.float32)
